# revision 25
# baseline (speedup 1.0000x reference)
"""Trainium2 Bass kernel for the attention-LSTM decoder (LAS-style).

Sharding: data-parallel over batch N=256 -> 32 per core across 8 cores.
Per-core layout is feature-major (features on SBUF partitions, batch in the
free dimension).  The 250-step recurrence runs fully unrolled on-device.

Structure (v2 of the kernel; rebuilt around trace measurements showing the
PE is weight-load bound and ~25% idle during serial ACT/DVE windows):
  - Activations via the Tanh table only (sigmoid(x) = 0.5 + 0.5*tanh(x/2),
    input scales folded into host-side weights, output affines fused into
    DVE scalar_tensor_tensor ops).  State is stored as H = 2h / S = 2c.
    exp/tanh live in one ACT table -> no table reloads.
  - Attention uses single-column stationaries: for sample n the energy
    matmul is keys[:, n]^T @ h2[:, n:n+1] written to ONE psum partition
    (scat(n) = 32*(n//8) + n%8, spreading samples across the four PE
    column-tiles).  No one-hot masks, no h2m staging; chunks of the
    encoder axis are merged into one long-stream matmul per sample with
    compile-time exact lengths (slot-sorted by length on the host).
  - Same trick for context: per (sample, chunk) a [128,1] stationary of
    masked-exp attn weights against the values block; the ones-column of
    values yields the softmax denominator.
  - Scheduling: W1e@x(t+1) runs under tanh/DVE of gates1; Whh1@h1 runs
    under the lstm2 tanh window; the logits h2-half runs under exp; the
    energy/exp/transpose/mask/ctx pipeline is split at t=256 so ACT and
    PE overlap; logits bias-add + output DMA live on the Pool engine.
"""

import functools
import sys

for _p in ("/opt/trn_rl_repo",):
    if _p not in sys.path:
        sys.path.insert(0, _p)

import numpy as np
import ml_dtypes

import concourse.bass as bass
import concourse.tile as tile
from concourse import bacc, mybir
from concourse import bass_utils
from concourse.masks import make_identity

dt = mybir.dt
AF = mybir.ActivationFunctionType
ALU = mybir.AluOpType

NCORES = 8
NB = 32            # batch per core
T = 400            # encoder length
NCH = 4            # time chunks of 128
KS = 128
VS = 128
VSP = VS + 1       # values + ones column (softmax denominator)
H = 512
G1 = 2048          # 4*H
G2 = 512           # 4*KS
TL = 250
VOC = 1000
VOCP = 1024

F16 = dt.float16
F32 = dt.float32


def _scat(n):
    return 32 * (n // 8) + n % 8


def _scat_ap(t):
    """AP over the 32 'scattered' columns {32j+m : j<4, m<8} of a 128-col
    feature-major tile, ordered n = 8j+m."""
    a = t[:]
    return bass.AP(a.tensor, a.offset, [a.ap[0], [32, 4], [1, 8]])


def _emit(tc, io, tl, lmax, hb1, hb2):
    nc = tc.nc
    pools = []
    _scopes = []

    def S(name):
        _scopes.append((name, nc.enter_named_scope(name, False)[0]))

    def E():
        n, i = _scopes.pop()
        nc.leave_named_scope(n, i, False)

    def pool(name, bufs, space="SBUF"):
        p = tc.alloc_tile_pool(name=name, bufs=bufs, space=space)
        pools.append(p)
        return p

    maxL = max(lmax)
    nch = (maxL + 127) // 128

    const = pool("const", 1)
    state = pool("state", 1)
    gate_p = pool("gate", 2)
    ls_p = pool("ls", 2)
    xet_p = pool("xet", 3)
    g1_p = pool("g1", 2, space="PSUM")
    pstate = pool("pstate", 1, space="PSUM")

    # ---- resident constants ----
    def load_const(key, shape, dtype):
        t = const.tile(shape, dtype, name=key + "_sb")
        nc.sync.dma_start(t[:], io[key].ap())
        return t

    w1e_sb = load_const("w1e", [128, 4 * G1], F16)
    keys_sb = load_const("keys", [128, NB * T], F16)
    vals_sb = load_const("vals", [128, NCH * NB * VSP], F16)
    whh1_sb = load_const("whh1", [128, 4 * G1], F16)
    w1c_sb = load_const("w1c", [128, G1], F16)
    w2i_sb = load_const("w2i", [128, 4 * G2], F16)
    w2h_sb = load_const("w2h", [128, G2], F16)
    wout_sb = load_const("wout", [128, 2 * VOCP], F16)
    bout_sb = load_const("bout", [128, 8], F32)
    # maskind[t', c*256 + n*8 + m] = (m==n%8) * (c*128+t' < len_n)
    maskind_sb = load_const("maskind", [128, NCH * 256], F16)
    if hb1:
        b1_sb = load_const("b1", [128, 16], F32)
    if hb2:
        b2_sb = load_const("b2", [128, 4], F32)

    ident_bf = const.tile([128, 128], F16, name="ident_bf")
    make_identity(nc, ident_bf[:])
    # mask8[p, n*8+m] = 1.0 iff m == n%8 (same on every partition)
    mask8 = const.tile([128, 256], F32, name="mask8")
    nc.gpsimd.memset(mask8[:], 0.0)
    nc.gpsimd.affine_select(
        out=mask8[:], in_=mask8[:],
        compare_op=ALU.not_equal, fill=1.0, base=0,
        pattern=[[0, 4], [-1, 8], [1, 8]], channel_multiplier=0,
    )
    mask8_32x8 = bass.AP(mask8[:].tensor, mask8[:].offset,
                         [mask8[:].ap[0], [8, 32], [1, 8]])

    # ---- state ----
    # gw: [i f g o | s1] (tnh output cols 0:512, S1 = 2*c1 cols 512:640)
    gw = state.tile([128, 640], F32, name="gw")
    gw2 = state.tile([128, 160], F32, name="gw2")   # [i f g o | s2], 32 each
    h1b = state.tile([128, 128], F16, name="h1b")   # H1 = 2*h1
    h2b = state.tile([128, 32], F16, name="h2b")    # H2 = 2*h2
    ctxb = state.tile([128, 128], F16, name="ctxb")  # scattered cols
    exp_t = state.tile([128, 512], F16, name="exp_t")
    attnTM = state.tile([128, NCH * 256], F16, name="attnTM")
    h2m = state.tile([128, 256], F16, name="h2m")
    rcp = state.tile([128, 1], F32, name="rcp")

    for tl_ in (gw, gw2):
        nc.vector.memset(tl_[:], 0.0)
    for tl_ in (h1b, h2b, exp_t):
        nc.vector.memset(tl_[:], 0.0)
    nc.sync.dma_start(ctxb[:], io["ctx0"].ap())

    # persistent psum tiles (1:1 bank reuse across steps; stale regions are
    # finite and get masked after exp)
    ep = pstate.tile([128, 512], F32, name="ep")
    cp = pstate.tile([128, 512], F32, name="cp")
    lp_t = pstate.tile([128, 512], F32, name="lp")
    g2p_t = pstate.tile([128, 512], F32, name="g2p")
    etp = pstate.tile([128, 1024], F16, name="etp")  # expT 0:512 | ctxT 512:640
    dum = pstate.tile([128, 512], F32, name="dum")
    for tl_ in (ep, cp):
        nc.vector.memset(tl_[:], 0.0)
    # denominator col reads 1.0 on the 96 dead partitions (finite rcp)
    nc.vector.memset(cp[:, 128:129], 1.0)
    lp = lp_t[:, 0:256]
    g2p = g2p_t[:, 0:128]

    out_t = io["out"].ap().tensor     # [TL, 8, 128, 32] f16
    xemb_t = io["xemb"].ap().tensor   # [4, 128, TL*NB] f16

    def emit_xet_dma(t):
        xt = xet_p.tile([128, 4 * 32], F16, tag="xet")
        src = bass.AP(xemb_t, t * NB,
                      [[TL * NB, 128], [128 * TL * NB, 4], [1, 32]])
        nc.sync.dma_start(xt[:].rearrange("p (k b) -> p k b", k=4), src)
        return xt

    def filler(n):
        # keep the PE's DVFS p-state high through serial ACT/DVE windows
        for i in range(n):
            nc.tensor.matmul(dum[:], whh1_sb[:, 0:128], keys_sb[:, 0:512],
                             start=(i == 0), stop=(i == n - 1),
                             skip_group_check=True)

    def emit_w1e(g1t, xt):
        # g1 partial: W1e @ xemb(t).  ONE start for the whole bank: the
        # pend covers all 2KB, each region's first write resets it, later
        # Whh1/W1c matmuls accumulate.
        S("w1e")
        for m in range(16):
            o = m * 32
            for k in range(4):
                nc.tensor.matmul(
                    g1t[:, o:o + 32],
                    w1e_sb[:, k * G1 + m * 128: k * G1 + (m + 1) * 128],
                    xt[:, k * 32:(k + 1) * 32],
                    start=(m == 0 and k == 0), stop=False,
                    skip_group_check=True)
        E()

    def emit_whh1(g1t):
        S("whh1")
        for m in range(16):
            o = m * 32
            for k in range(4):
                nc.tensor.matmul(
                    g1t[:, o:o + 32],
                    whh1_sb[:, k * G1 + m * 128: k * G1 + (m + 1) * 128],
                    h1b[:, k * 32:(k + 1) * 32],
                    start=False, stop=False, skip_group_check=True)
        E()

    def emit_w1c(g1t):
        S("w1c")
        ctx_rhs = _scat_ap(ctxb)
        for m in range(16):
            o = m * 32
            nc.tensor.matmul(
                g1t[:, o:o + 32], w1c_sb[:, m * 128:(m + 1) * 128],
                ctx_rhs, start=False, stop=(m == 15), skip_group_check=True)
        E()

    def pair_ap(tile_, off, stride, w):
        a = tile_[:]
        return bass.AP(a.tensor, a.offset + off, [a.ap[0], [stride, 2], [1, w]])

    # per-quadrant chain bounds: energy half-1 stop (if no half-2 in the
    # quadrant), half-2 stop; ctx first/last (c, mm) pairs
    e_stop1 = {}
    e_stop2 = {}
    for j in range(4):
        long_mm = [mm for mm in range(8) if lmax[8 * j + mm] > 256]
        e_stop1[j] = 7 if not long_mm else None
        if long_mm:
            e_stop2[j] = long_mm[-1]
    cmax = [(lmax[n] + 127) // 128 for n in range(NB)]
    c_pairs = {j: [(c, mm) for c in range(nch) for mm in range(8)
                   if c < cmax[8 * j + mm]] for j in range(4)}
    c_first = {j: p[0] for j, p in c_pairs.items()}
    c_last = {j: p[-1] for j, p in c_pairs.items()}

    # ---- prologue ----
    xcur = emit_xet_dma(0)
    xnext = emit_xet_dma(1) if tl > 1 else None
    g1cur = g1_p.tile([128, 512], F32, tag="g1")
    emit_w1e(g1cur, xcur)
    emit_whh1(g1cur)
    emit_w1c(g1cur)

    # ---- the recurrence ----
    for t in range(tl):
        if t + 2 < tl:
            xnext2 = emit_xet_dma(t + 2)

        # PE: next step's W1e chain fills the gates1 ACT/DVE window
        if t + 1 < tl:
            g1next = g1_p.tile([128, 512], F32, tag="g1")
            emit_w1e(g1next, xnext)

        S("gates1")
        if hb1:
            g1sb = gate_p.tile([128, 512], F32, tag="g1sb")
            b1_bc = bass.AP(b1_sb[:].tensor, b1_sb[:].offset,
                            [b1_sb[:].ap[0], [1, 16], [0, 32]])
            nc.vector.tensor_add(
                g1sb[:].rearrange("p (m b) -> p m b", m=16),
                g1cur[:].rearrange("p (m b) -> p m b", m=16), b1_bc)
            nc.scalar.activation(gw[:, 0:512], g1sb[:], AF.Tanh)
        else:
            nc.scalar.activation(gw[:, 0:512], g1cur[:], AF.Tanh)
        # a2 = (i+1)*g ; a1 = (f+1)*S1 ; S1 = a1*0.5 + a2
        aa = gate_p.tile([128, 256], F32, tag="aa")
        nc.vector.scalar_tensor_tensor(
            aa[:, 0:128], gw[:, 0:128], 1.0, gw[:, 256:384],
            ALU.add, ALU.mult)
        nc.vector.scalar_tensor_tensor(
            aa[:, 128:256], gw[:, 128:256], 1.0, gw[:, 512:640],
            ALU.add, ALU.mult)
        nc.vector.scalar_tensor_tensor(
            gw[:, 512:640], aa[:, 128:256], 0.5, aa[:, 0:128],
            ALU.mult, ALU.add)
        th = gate_p.tile([128, 128], F32, tag="th")
        nc.scalar.activation(th[:], gw[:, 512:640], AF.Tanh, scale=0.5)
        nc.vector.scalar_tensor_tensor(
            h1b[:], gw[:, 384:512], 1.0, th[:], ALU.add, ALU.mult)
        E()

        S("lstm2")
        for m in range(4):
            o = m * 32
            nc.tensor.matmul(
                g2p[:, o:o + 32], w2h_sb[:, m * 128:(m + 1) * 128],
                h2b[:], start=True, stop=False, skip_group_check=True)
            for k in range(4):
                nc.tensor.matmul(
                    g2p[:, o:o + 32],
                    w2i_sb[:, k * G2 + m * 128: k * G2 + (m + 1) * 128],
                    h1b[:, k * 32:(k + 1) * 32],
                    start=False, stop=(k == 3), skip_group_check=True)
        if hb2:
            g2sb = gate_p.tile([128, 128], F32, tag="g2sb")
            b2_bc = bass.AP(b2_sb[:].tensor, b2_sb[:].offset,
                            [b2_sb[:].ap[0], [1, 4], [0, 32]])
            nc.vector.tensor_add(
                g2sb[:].rearrange("p (m b) -> p m b", m=4),
                g2p[:].rearrange("p (m b) -> p m b", m=4), b2_bc)
            nc.scalar.activation(gw2[:, 0:128], g2sb[:], AF.Tanh)
        else:
            nc.scalar.activation(gw2[:, 0:128], g2p[:], AF.Tanh)
        aa2 = gate_p.tile([128, 64], F32, tag="aa2")
        nc.vector.scalar_tensor_tensor(
            aa2[:, 0:32], gw2[:, 0:32], 1.0, gw2[:, 64:96],
            ALU.add, ALU.mult)
        nc.vector.scalar_tensor_tensor(
            aa2[:, 32:64], gw2[:, 32:64], 1.0, gw2[:, 128:160],
            ALU.add, ALU.mult)
        nc.vector.scalar_tensor_tensor(
            gw2[:, 128:160], aa2[:, 32:64], 0.5, aa2[:, 0:32],
            ALU.mult, ALU.add)
        th2 = gate_p.tile([128, 32], F32, tag="th2")
        nc.scalar.activation(th2[:], gw2[:, 128:160], AF.Tanh, scale=0.5)
        nc.vector.scalar_tensor_tensor(
            h2b[:], gw2[:, 96:128], 1.0, th2[:], ALU.add, ALU.mult)
        h2bc = bass.AP(h2b[:].tensor, h2b[:].offset,
                       [h2b[:].ap[0], [1, 32], [0, 8]])
        nc.vector.tensor_mul(
            h2m[:].rearrange("p (n m) -> p n m", m=8), h2bc, mask8_32x8)
        E()

        # PE: Whh1 @ h1(t) for step t+1 fills the lstm2 tanh/DVE window
        if t + 1 < tl:
            emit_whh1(g1next)

        S("energy")
        # one long-stream matmul per (sample, half) with one-hot-masked h2
        # columns writing that sample's psum row; the encoder axis is split
        # at col 256 so exp of the first half overlaps the tail.  Chains
        # accumulate per quadrant (column tile).
        for n in range(NB):
            L = lmax[n]
            j = n // 8
            mm = n % 8
            L1 = min(L, 256)
            nc.tensor.matmul(
                ep[32 * j:32 * j + 8, 0:L1], h2m[:, n * 8:(n + 1) * 8],
                keys_sb[:, n * T: n * T + L1],
                start=(mm == 0), stop=(mm == e_stop1[j]),
                tile_position=(0, 32 * j), skip_group_check=True)
        E()

        S("logits")
        # h2 half of the logits: independent PE work under the exp window.
        # ONE start for the whole bank -- a per-region start would re-pend
        # the earlier regions' bytes and the ctx-half accumulation below
        # would then overwrite instead of accumulate.
        for mo in range(8):
            nc.tensor.matmul(lp[:, mo * 32:(mo + 1) * 32],
                             wout_sb[:, mo * 128:(mo + 1) * 128],
                             h2b[:], start=(mo == 0), stop=False,
                             skip_group_check=True)
        E()

        S("softmax")
        nc.scalar.activation(exp_t[:, 0:256], ep[:, 0:256], AF.Exp)
        E()

        S("energy2")
        for n in range(NB):
            L = lmax[n]
            if L <= 256:
                continue
            j = n // 8
            mm = n % 8
            nc.tensor.matmul(
                ep[32 * j:32 * j + 8, 256:L], h2m[:, n * 8:(n + 1) * 8],
                keys_sb[:, n * T + 256: n * T + L],
                start=False, stop=(mm == e_stop2.get(j)),
                tile_position=(0, 32 * j), skip_group_check=True)
        E()
        if maxL > 256:
            S("softmax2")
            nc.scalar.activation(exp_t[:, 256:512], ep[:, 256:512], AF.Exp)
            E()

        S("transp")
        for c in range(nch):
            nc.tensor.transpose(etp[:, c * 128:(c + 1) * 128],
                                exp_t[:, c * 128:(c + 1) * 128], ident_bf[:])
            src = bass.AP(etp[:].tensor, etp[:].offset + c * 128,
                          [etp[:].ap[0], [32, 4], [1, 8], [0, 8]])
            nc.vector.tensor_mul(
                attnTM[:, c * 256:(c + 1) * 256].rearrange(
                    "p (j mm m) -> p j mm m", j=4, mm=8),
                src,
                maskind_sb[:, c * 256:(c + 1) * 256].rearrange(
                    "p (j mm m) -> p j mm m", j=4, mm=8))
            if c == 1 or c == nch - 1:
                # context accumulation for the chunks transposed so far
                S("ctx")
                c_lo = 0 if c <= 1 else 2
                for cc in range(c_lo, c + 1):
                    for n in range(NB):
                        if cc >= cmax[n]:
                            continue
                        j = n // 8
                        mm = n % 8
                        nc.tensor.matmul(
                            cp[32 * j:32 * j + 8, 0:VSP],
                            attnTM[:, cc * 256 + n * 8: cc * 256 + (n + 1) * 8],
                            vals_sb[:, (cc * NB + n) * VSP:
                                    (cc * NB + n + 1) * VSP],
                            start=((cc, mm) == c_first[j]),
                            stop=((cc, mm) == c_last[j]),
                            tile_position=(0, 32 * j),
                            skip_group_check=True)
                E()
        E()

        S("ctxfin")
        nc.vector.reciprocal(rcp[:], cp[:, 128:129])
        ctxbm = ls_p.tile([128, 128], F16, tag="ctxbm")
        nc.vector.tensor_scalar_mul(ctxbm[:], cp[:, 0:128], rcp[:])
        filler(2)
        nc.tensor.transpose(etp[:, 512:640], ctxbm[:], ident_bf[:])
        nc.vector.tensor_copy(ctxb[:], etp[:, 512:640])
        E()

        if t + 1 < tl:
            emit_w1c(g1next)

        S("logits2")
        ctx_rhs2 = _scat_ap(ctxb)
        for mo in range(8):
            nc.tensor.matmul(lp[:, mo * 32:(mo + 1) * 32],
                             wout_sb[:, VOCP + mo * 128: VOCP + (mo + 1) * 128],
                             ctx_rhs2, start=False, stop=True,
                             skip_group_check=True)
        ls = ls_p.tile([128, 256], F16, tag="ls")
        bout_bc = bass.AP(bout_sb[:].tensor, bout_sb[:].offset,
                          [bout_sb[:].ap[0], [1, 8], [0, 32]])
        nc.vector.tensor_add(
            ls[:].rearrange("p (mo b) -> p mo b", mo=8),
            lp[:, 0:256].rearrange("p (mo b) -> p mo b", mo=8), bout_bc)
        dst = bass.AP(out_t, t * 8 * 128 * 32,
                      [[32, 128], [128 * 32, 8], [1, 32]])
        nc.gpsimd.dma_start(dst, ls[:].rearrange("p (mo b) -> p mo b", mo=8))
        E()

        if t + 1 < tl:
            g1cur = g1next
            xcur = xnext
            xnext = xnext2 if t + 2 < tl else None

    for p in reversed(pools):
        p.release()


@functools.lru_cache(maxsize=4)
def _build(tl, lmax, hb1, hb2):
    nc = bacc.Bacc("TRN2", target_bir_lowering=False, debug=False)
    io = {}
    io["keys"] = nc.dram_tensor("keys", [128, NB * T], F16, kind="ExternalInput")
    io["vals"] = nc.dram_tensor("vals", [128, NCH * NB * VSP], F16, kind="ExternalInput")
    io["xemb"] = nc.dram_tensor("xemb", [4, 128, TL * NB], F16, kind="ExternalInput")
    io["w1e"] = nc.dram_tensor("w1e", [128, 4 * G1], F16, kind="ExternalInput")
    io["whh1"] = nc.dram_tensor("whh1", [128, 4 * G1], F16, kind="ExternalInput")
    io["w1c"] = nc.dram_tensor("w1c", [128, G1], F16, kind="ExternalInput")
    io["w2i"] = nc.dram_tensor("w2i", [128, 4 * G2], F16, kind="ExternalInput")
    io["w2h"] = nc.dram_tensor("w2h", [128, G2], F16, kind="ExternalInput")
    io["wout"] = nc.dram_tensor("wout", [128, 2 * VOCP], F16, kind="ExternalInput")
    io["b1"] = nc.dram_tensor("b1", [128, 16], F32, kind="ExternalInput")
    io["b2"] = nc.dram_tensor("b2", [128, 4], F32, kind="ExternalInput")
    io["bout"] = nc.dram_tensor("bout", [128, 8], F32, kind="ExternalInput")
    io["ctx0"] = nc.dram_tensor("ctx0", [128, 128], F16, kind="ExternalInput")
    io["maskind"] = nc.dram_tensor("maskind", [128, NCH * 256], F16, kind="ExternalInput")
    io["out"] = nc.dram_tensor("out", [TL, 8, 128, 32], F16, kind="ExternalOutput")

    with tile.TileContext(nc) as tc:
        _emit(tc, io, tl, lmax, hb1, hb2)
    nc.compile()
    return nc


def _bf(x):
    return np.asarray(x, np.float32).astype(np.float16)


def _scat_perm():
    n = np.arange(NB)
    return 32 * (n // 8) + n % 8


def prep_inputs(key, values, lens, text, emb, W_ih1, W_hh1, b_ih1, b_hh1,
                W_ih2, W_hh2, b_ih2, b_hh2, W_out, b_out):
    key = np.asarray(key, np.float32)
    values = np.asarray(values, np.float32)
    lens = np.asarray(lens).astype(np.int64)
    text = np.asarray(text).astype(np.int64)
    emb = np.asarray(emb, np.float32)
    W_ih1 = np.asarray(W_ih1, np.float32)
    W_hh1 = np.asarray(W_hh1, np.float32)
    W_ih2 = np.asarray(W_ih2, np.float32)
    W_hh2 = np.asarray(W_hh2, np.float32)
    W_out = np.asarray(W_out, np.float32)
    b1 = np.asarray(b_ih1, np.float32) + np.asarray(b_hh1, np.float32)
    b2 = np.asarray(b_ih2, np.float32) + np.asarray(b_hh2, np.float32)
    b_out = np.asarray(b_out, np.float32)

    perm = _scat_perm()

    # sigmoid-via-tanh input scales (i, f, o rows) and the H = 2h / S = 2c
    # state-scaling compensation on consumer weights
    rs1 = np.ones((4 * H, 1), np.float32)
    rs1[0:2 * H] = 0.5          # i, f
    rs1[3 * H:4 * H] = 0.5      # o
    rs2 = np.ones((4 * KS, 1), np.float32)
    rs2[0:2 * KS] = 0.5
    rs2[3 * KS:4 * KS] = 0.5

    W1 = W_ih1 * rs1
    Wh1 = W_hh1 * rs1 * 0.5
    W2i = W_ih2 * rs2 * 0.5
    W2h = W_hh2 * rs2 * 0.5
    b1s = b1 * rs1.ravel()
    b2s = b2 * rs2.ravel()

    shared = {}
    w1T = np.ascontiguousarray(W1.T)  # (640, 2048)
    shared["w1e"] = _bf(w1T[:H].reshape(4, 128, G1).transpose(1, 0, 2).reshape(128, 4 * G1))
    shared["w1c"] = _bf(w1T[H:])
    shared["whh1"] = _bf(Wh1.T.reshape(4, 128, G1).transpose(1, 0, 2).reshape(128, 4 * G1))
    shared["w2i"] = _bf(W2i.T.reshape(4, 128, G2).transpose(1, 0, 2).reshape(128, 4 * G2))
    shared["w2h"] = _bf(np.ascontiguousarray(W2h.T))
    woutp = np.zeros((VOCP, KS + VS), np.float32)
    woutp[:VOC] = W_out
    woutp[:, :KS] *= 0.5        # h2 = H2/2
    shared["wout"] = _bf(woutp.T.reshape(2, 128, VOCP).transpose(1, 0, 2).reshape(128, 2 * VOCP))
    shared["b1"] = np.ascontiguousarray(b1s.reshape(16, 128).T)
    shared["b2"] = np.ascontiguousarray(b2s.reshape(4, 128).T)
    boutp = np.zeros((VOCP,), np.float32)
    boutp[:VOC] = b_out
    shared["bout"] = np.ascontiguousarray(boutp.reshape(8, 128).T)

    # sort batches into slots by length (ascending) per core; the slot-wise
    # max over cores defines the compile-time length profile
    lens_c = lens.reshape(NCORES, NB)
    orders = [np.argsort(lens_c[c], kind="stable") for c in range(NCORES)]
    slot_lens = np.stack([lens_c[c][orders[c]] for c in range(NCORES)])
    lmax = tuple(int(v) for v in slot_lens.max(axis=0))

    in_maps = []
    for core in range(NCORES):
        sl = slice(core * NB, (core + 1) * NB)
        order = orders[core]
        keyc = key[:, sl, :][:, order, :]
        valc = values[:, sl, :][:, order, :]
        lensc = lens[sl][order]
        textc = text[sl][order]

        m = dict(shared)
        # zero the invalid (t >= len) key rows (masked energies become 0)
        # and fold the H2 = 2*h2 compensation into the keys
        kz = keyc * 0.5 * (np.arange(T)[:, None, None] < lensc[None, :, None])
        m["keys"] = _bf(np.ascontiguousarray(
            kz.transpose(2, 1, 0)).reshape(128, NB * T))
        vp = np.zeros((NCH * 128, NB, VSP), np.float32)
        vp[:T, :, :VS] = valc
        vp[:, :, VS] = 1.0       # ones column -> softmax denominator
        m["vals"] = _bf(np.ascontiguousarray(
            vp.reshape(NCH, 128, NB * VSP).transpose(1, 0, 2)).reshape(
                128, NCH * NB * VSP))
        embs = emb[textc]                       # (32, TL, H)
        m["xemb"] = _bf(np.ascontiguousarray(
            embs.transpose(2, 1, 0)).reshape(4, 128, TL * NB))
        ctx0 = valc.mean(axis=0)                # (32, VS)
        c0 = np.zeros((128, 128), np.float32)
        c0[:, perm] = ctx0.T
        m["ctx0"] = _bf(c0)
        # maskind[t', c*256 + n*8 + m] = (m==n%8) * (c*128+t' < len_n)
        ind = (np.arange(NCH * 128)[None, :] < lensc[:, None]).astype(np.float32)
        mi = np.zeros((128, NCH, 32, 8), np.float32)
        nn = np.arange(NB)
        mi[:, :, nn, nn % 8] = ind.reshape(NB, NCH, 128).transpose(2, 1, 0)
        m["maskind"] = _bf(mi.reshape(128, NCH * 256))
        in_maps.append(m)
    return in_maps, orders, lmax, b1s, b2s


def kernel(key, values, lens, text, emb, W_ih1, W_hh1, b_ih1, b_hh1,
           W_ih2, W_hh2, b_ih2, b_hh2, W_out, b_out,
           _trace=False, _tl=TL):
    in_maps, orders, lmax, b1s, b2s = prep_inputs(
        key, values, lens, text, emb, W_ih1, W_hh1, b_ih1, b_hh1,
        W_ih2, W_hh2, b_ih2, b_hh2, W_out, b_out)
    hb1 = bool(np.any(b1s))
    hb2 = bool(np.any(b2s))
    if __import__('os').environ.get('NOTRIM'):
        lmax = tuple(T for _ in lmax)
    nc = _build(_tl, lmax, hb1, hb2)
    res = bass_utils.run_bass_kernel_spmd(
        nc, in_maps, core_ids=list(range(NCORES)), trace=_trace)
    kernel._last_results = res

    full = np.zeros((NCORES * NB, TL, VOC), np.float32)
    for core in range(NCORES):
        o = np.asarray(res.results[core]["out"]).astype(np.float32)
        o = o.reshape(TL, VOCP, 32)
        full[core * NB + orders[core]] = o[:, :VOC, :].transpose(2, 0, 1)
    return full


# revision 28
# speedup vs baseline: 1.0343x; 1.0343x over previous
"""Trainium2 Bass kernel for the attention-LSTM decoder (LAS-style).

Sharding: data-parallel over batch N=256 -> 32 per core across 8 cores.
Per-core layout is feature-major (features on SBUF partitions, batch in the
free dimension).  The 250-step recurrence runs fully unrolled on-device.

Structure (v2 of the kernel; rebuilt around trace measurements showing the
PE is weight-load bound and ~25% idle during serial ACT/DVE windows):
  - Activations via the Tanh table only (sigmoid(x) = 0.5 + 0.5*tanh(x/2),
    input scales folded into host-side weights, output affines fused into
    DVE scalar_tensor_tensor ops).  State is stored as H = 2h / S = 2c.
    exp/tanh live in one ACT table -> no table reloads.
  - Attention uses single-column stationaries: for sample n the energy
    matmul is keys[:, n]^T @ h2[:, n:n+1] written to ONE psum partition
    (scat(n) = 32*(n//8) + n%8, spreading samples across the four PE
    column-tiles).  No one-hot masks, no h2m staging; chunks of the
    encoder axis are merged into one long-stream matmul per sample with
    compile-time exact lengths (slot-sorted by length on the host).
  - Same trick for context: per (sample, chunk) a [128,1] stationary of
    masked-exp attn weights against the values block; the ones-column of
    values yields the softmax denominator.
  - Scheduling: W1e@x(t+1) runs under tanh/DVE of gates1; Whh1@h1 runs
    under the lstm2 tanh window; the logits h2-half runs under exp; the
    energy/exp/transpose/mask/ctx pipeline is split at t=256 so ACT and
    PE overlap; logits bias-add + output DMA live on the Pool engine.
"""

import functools
import sys

for _p in ("/opt/trn_rl_repo",):
    if _p not in sys.path:
        sys.path.insert(0, _p)

import numpy as np
import ml_dtypes

import concourse.bass as bass
import concourse.tile as tile
from concourse import bacc, mybir
from concourse import bass_utils
from concourse.masks import make_identity

dt = mybir.dt
AF = mybir.ActivationFunctionType
ALU = mybir.AluOpType

NCORES = 8
NB = 32            # batch per core
T = 400            # encoder length
NCH = 4            # time chunks of 128
KS = 128
VS = 128
VSP = VS + 1       # values + ones column (softmax denominator)
H = 512
G1 = 2048          # 4*H
G2 = 512           # 4*KS
TL = 250
VOC = 1000
VOCP = 1024

F16 = dt.float16
F32 = dt.float32


def _scat(n):
    return 32 * (n // 8) + n % 8


def _scat_ap(t):
    """AP over the 32 'scattered' columns {32j+m : j<4, m<8} of a 128-col
    feature-major tile, ordered n = 8j+m."""
    a = t[:]
    return bass.AP(a.tensor, a.offset, [a.ap[0], [32, 4], [1, 8]])


def _emit(tc, io, tl, lmax, hb1, hb2):
    nc = tc.nc
    pools = []
    _scopes = []

    def S(name):
        _scopes.append((name, nc.enter_named_scope(name, False)[0]))

    def E():
        n, i = _scopes.pop()
        nc.leave_named_scope(n, i, False)

    def pool(name, bufs, space="SBUF"):
        p = tc.alloc_tile_pool(name=name, bufs=bufs, space=space)
        pools.append(p)
        return p

    maxL = max(lmax)
    nch = (maxL + 127) // 128

    const = pool("const", 1)
    state = pool("state", 1)
    gate_p = pool("gate", 2)
    ls_p = pool("ls", 2)
    xet_p = pool("xet", 3)
    g1_p = pool("g1", 2, space="PSUM")
    pstate = pool("pstate", 1, space="PSUM")

    # ---- resident constants ----
    def load_const(key, shape, dtype):
        t = const.tile(shape, dtype, name=key + "_sb")
        nc.sync.dma_start(t[:], io[key].ap())
        return t

    w1e_sb = load_const("w1e", [128, 4 * G1], F16)
    keys_sb = load_const("keys", [128, NB * T], F16)
    vals_sb = load_const("vals", [128, NCH * NB * VSP], F16)
    whh1_sb = load_const("whh1", [128, 4 * G1], F16)
    w1c_sb = load_const("w1c", [128, G1], F16)
    w2i_sb = load_const("w2i", [128, 4 * G2], F16)
    w2h_sb = load_const("w2h", [128, G2], F16)
    wout_sb = load_const("wout", [128, 2 * VOCP], F16)
    bout_sb = load_const("bout", [128, 8], F32)
    # maskind[t', c*256 + n*8 + m] = (m==n%8) * (c*128+t' < len_n)
    maskind_sb = load_const("maskind", [128, NCH * 256], F16)
    if hb1:
        b1_sb = load_const("b1", [128, 16], F32)
    if hb2:
        b2_sb = load_const("b2", [128, 4], F32)

    ident_bf = const.tile([128, 128], F16, name="ident_bf")
    make_identity(nc, ident_bf[:])
    # mask8[p, n*8+m] = 1.0 iff m == n%8 (same on every partition)
    mask8 = const.tile([128, 256], F32, name="mask8")
    nc.gpsimd.memset(mask8[:], 0.0)
    nc.gpsimd.affine_select(
        out=mask8[:], in_=mask8[:],
        compare_op=ALU.not_equal, fill=1.0, base=0,
        pattern=[[0, 4], [-1, 8], [1, 8]], channel_multiplier=0,
    )
    mask8_32x8 = bass.AP(mask8[:].tensor, mask8[:].offset,
                         [mask8[:].ap[0], [8, 32], [1, 8]])

    # ---- state ----
    # gw: [i f g o | s1] (tnh output cols 0:512, S1 = 2*c1 cols 512:640)
    gw = state.tile([128, 640], F32, name="gw")
    gw2 = state.tile([128, 160], F32, name="gw2")   # [i f g o | s2], 32 each
    h1b = state.tile([128, 128], F16, name="h1b")   # H1 = 2*h1
    h2b = state.tile([128, 32], F16, name="h2b")    # H2 = 2*h2
    ctxb = state.tile([128, 128], F16, name="ctxb")  # scattered cols
    exp_t = state.tile([128, 512], F16, name="exp_t")
    attnTM = state.tile([128, NCH * 256], F16, name="attnTM")
    h2m = state.tile([128, 256], F16, name="h2m")
    rcp = state.tile([128, 1], F32, name="rcp")

    for tl_ in (gw, gw2):
        nc.vector.memset(tl_[:], 0.0)
    for tl_ in (h1b, h2b, exp_t):
        nc.vector.memset(tl_[:], 0.0)
    nc.sync.dma_start(ctxb[:], io["ctx0"].ap())

    # persistent psum tiles (1:1 bank reuse across steps; stale regions are
    # finite and get masked after exp)
    ep = pstate.tile([128, 512], F32, name="ep")
    cp = pstate.tile([128, 512], F32, name="cp")
    lp_t = pstate.tile([128, 512], F32, name="lp")
    g2p_t = pstate.tile([128, 512], F32, name="g2p")
    etp = pstate.tile([128, 1024], F16, name="etp")  # expT 0:512 | ctxT 512:640
    dum = pstate.tile([128, 512], F32, name="dum")
    for tl_ in (ep, cp):
        nc.vector.memset(tl_[:], 0.0)
    # denominator col reads 1.0 on the 96 dead partitions (finite rcp)
    nc.vector.memset(cp[:, 128:129], 1.0)
    lp = lp_t[:, 0:256]
    g2p = g2p_t[:, 0:128]

    out_t = io["out"].ap().tensor     # [TL, 8, 128, 32] f16
    xemb_t = io["xemb"].ap().tensor   # [4, 128, TL*NB] f16

    def emit_xet_dma(t):
        xt = xet_p.tile([128, 4 * 32], F16, tag="xet")
        src = bass.AP(xemb_t, t * NB,
                      [[TL * NB, 128], [128 * TL * NB, 4], [1, 32]])
        nc.sync.dma_start(xt[:].rearrange("p (k b) -> p k b", k=4), src)
        return xt

    def filler(n):
        # keep the PE's DVFS p-state high through serial ACT/DVE windows
        for i in range(n):
            nc.tensor.matmul(dum[:], whh1_sb[:, 0:128], keys_sb[:, 0:512],
                             start=(i == 0), stop=(i == n - 1),
                             skip_group_check=True)

    def emit_w1e(g1t, xt):
        # g1 partial: W1e @ xemb(t).  ONE start for the whole bank: the
        # pend covers all 2KB, each region's first write resets it, later
        # Whh1/W1c matmuls accumulate.
        S("w1e")
        for m in range(16):
            o = m * 32
            for k in range(4):
                nc.tensor.matmul(
                    g1t[:, o:o + 32],
                    w1e_sb[:, k * G1 + m * 128: k * G1 + (m + 1) * 128],
                    xt[:, k * 32:(k + 1) * 32],
                    start=(m == 0 and k == 0), stop=False,
                    skip_group_check=True)
        E()

    def emit_whh1(g1t):
        S("whh1")
        for m in range(16):
            o = m * 32
            for k in range(4):
                nc.tensor.matmul(
                    g1t[:, o:o + 32],
                    whh1_sb[:, k * G1 + m * 128: k * G1 + (m + 1) * 128],
                    h1b[:, k * 32:(k + 1) * 32],
                    start=False, stop=False, skip_group_check=True)
        E()

    def emit_w1c(g1t):
        S("w1c")
        ctx_rhs = _scat_ap(ctxb)
        for m in range(16):
            o = m * 32
            nc.tensor.matmul(
                g1t[:, o:o + 32], w1c_sb[:, m * 128:(m + 1) * 128],
                ctx_rhs, start=False, stop=(m == 15), skip_group_check=True)
        E()

    def pair_ap(tile_, off, stride, w):
        a = tile_[:]
        return bass.AP(a.tensor, a.offset + off, [a.ap[0], [stride, 2], [1, w]])

    # per-quadrant chain bounds: energy half-1 stop (if no half-2 in the
    # quadrant), half-2 stop; ctx first/last (c, mm) pairs
    e_stop1 = {}
    e_stop2 = {}
    for j in range(4):
        long_mm = [mm for mm in range(8) if lmax[8 * j + mm] > 256]
        e_stop1[j] = 7 if not long_mm else None
        if long_mm:
            e_stop2[j] = long_mm[-1]
    cmax = [(lmax[n] + 127) // 128 for n in range(NB)]
    c_pairs = {j: [(c, mm) for c in range(nch) for mm in range(8)
                   if c < cmax[8 * j + mm]] for j in range(4)}
    c_first = {j: p[0] for j, p in c_pairs.items()}
    c_last = {j: p[-1] for j, p in c_pairs.items()}

    # ---- prologue ----
    xcur = emit_xet_dma(0)
    xnext = emit_xet_dma(1) if tl > 1 else None
    g1cur = g1_p.tile([128, 512], F32, tag="g1")
    emit_w1e(g1cur, xcur)
    emit_whh1(g1cur)
    emit_w1c(g1cur)

    # ---- the recurrence ----
    for t in range(tl):
        if t + 2 < tl:
            xnext2 = emit_xet_dma(t + 2)

        # PE: next step's W1e chain fills the gates1 ACT/DVE window
        if t + 1 < tl:
            g1next = g1_p.tile([128, 512], F32, tag="g1")
            emit_w1e(g1next, xnext)

        S("gates1")
        if hb1:
            g1sb = gate_p.tile([128, 512], F32, tag="g1sb")
            b1_bc = bass.AP(b1_sb[:].tensor, b1_sb[:].offset,
                            [b1_sb[:].ap[0], [1, 16], [0, 32]])
            nc.vector.tensor_add(
                g1sb[:].rearrange("p (m b) -> p m b", m=16),
                g1cur[:].rearrange("p (m b) -> p m b", m=16), b1_bc)
            nc.scalar.activation(gw[:, 0:512], g1sb[:], AF.Tanh)
        else:
            nc.scalar.activation(gw[:, 0:512], g1cur[:], AF.Tanh)
        # a2 = (i+1)*g ; a1 = (f+1)*S1 ; S1 = a1*0.5 + a2
        aa = gate_p.tile([128, 256], F32, tag="aa")
        nc.vector.scalar_tensor_tensor(
            aa[:, 0:128], gw[:, 0:128], 1.0, gw[:, 256:384],
            ALU.add, ALU.mult)
        nc.vector.scalar_tensor_tensor(
            aa[:, 128:256], gw[:, 128:256], 1.0, gw[:, 512:640],
            ALU.add, ALU.mult)
        nc.vector.scalar_tensor_tensor(
            gw[:, 512:640], aa[:, 128:256], 0.5, aa[:, 0:128],
            ALU.mult, ALU.add)
        th = gate_p.tile([128, 128], F32, tag="th")
        nc.scalar.activation(th[:], gw[:, 512:640], AF.Tanh, scale=0.5)
        nc.vector.scalar_tensor_tensor(
            h1b[:], gw[:, 384:512], 1.0, th[:], ALU.add, ALU.mult)
        E()

        S("lstm2")
        for m in range(4):
            o = m * 32
            nc.tensor.matmul(
                g2p[:, o:o + 32], w2h_sb[:, m * 128:(m + 1) * 128],
                h2b[:], start=True, stop=False, skip_group_check=True)
            for k in range(4):
                nc.tensor.matmul(
                    g2p[:, o:o + 32],
                    w2i_sb[:, k * G2 + m * 128: k * G2 + (m + 1) * 128],
                    h1b[:, k * 32:(k + 1) * 32],
                    start=False, stop=(k == 3), skip_group_check=True)
        if hb2:
            g2sb = gate_p.tile([128, 128], F32, tag="g2sb")
            b2_bc = bass.AP(b2_sb[:].tensor, b2_sb[:].offset,
                            [b2_sb[:].ap[0], [1, 4], [0, 32]])
            nc.vector.tensor_add(
                g2sb[:].rearrange("p (m b) -> p m b", m=4),
                g2p[:].rearrange("p (m b) -> p m b", m=4), b2_bc)
            nc.scalar.activation(gw2[:, 0:128], g2sb[:], AF.Tanh)
        else:
            nc.scalar.activation(gw2[:, 0:128], g2p[:], AF.Tanh)
        aa2 = gate_p.tile([128, 64], F32, tag="aa2")
        nc.vector.scalar_tensor_tensor(
            aa2[:, 0:32], gw2[:, 0:32], 1.0, gw2[:, 64:96],
            ALU.add, ALU.mult)
        nc.vector.scalar_tensor_tensor(
            aa2[:, 32:64], gw2[:, 32:64], 1.0, gw2[:, 128:160],
            ALU.add, ALU.mult)
        nc.vector.scalar_tensor_tensor(
            gw2[:, 128:160], aa2[:, 32:64], 0.5, aa2[:, 0:32],
            ALU.mult, ALU.add)
        th2 = gate_p.tile([128, 32], F32, tag="th2")
        nc.scalar.activation(th2[:], gw2[:, 128:160], AF.Tanh, scale=0.5)
        nc.vector.scalar_tensor_tensor(
            h2b[:], gw2[:, 96:128], 1.0, th2[:], ALU.add, ALU.mult)
        h2bc = bass.AP(h2b[:].tensor, h2b[:].offset,
                       [h2b[:].ap[0], [1, 32], [0, 8]])
        nc.vector.tensor_mul(
            h2m[:].rearrange("p (n m) -> p n m", m=8), h2bc, mask8_32x8)
        E()

        # PE: Whh1 @ h1(t) for step t+1 fills the lstm2 tanh/DVE window
        if t + 1 < tl:
            emit_whh1(g1next)

        S("energy")
        # one long-stream matmul per (sample, half) with one-hot-masked h2
        # columns writing that sample's psum row; the encoder axis is split
        # at col 256 so exp of the first half overlaps the tail.  Chains
        # accumulate per quadrant (column tile).
        for mm in range(8):
            for j in range(4):
                n = 8 * j + mm
                L = lmax[n]
                L1 = min(L, 256)
                nc.tensor.matmul(
                    ep[32 * j:32 * j + 8, 0:L1], h2m[:, n * 8:(n + 1) * 8],
                    keys_sb[:, n * T: n * T + L1],
                    start=(mm == 0), stop=(mm == e_stop1[j]),
                    tile_position=(0, 32 * j), skip_group_check=True)
        E()

        S("logits")
        # h2 half of the logits: independent PE work under the exp window.
        # ONE start for the whole bank -- a per-region start would re-pend
        # the earlier regions' bytes and the ctx-half accumulation below
        # would then overwrite instead of accumulate.
        for mo in range(8):
            nc.tensor.matmul(lp[:, mo * 32:(mo + 1) * 32],
                             wout_sb[:, mo * 128:(mo + 1) * 128],
                             h2b[:], start=(mo == 0), stop=False,
                             skip_group_check=True)
        E()

        S("softmax")
        nc.scalar.activation(exp_t[:, 0:256], ep[:, 0:256], AF.Exp)
        E()

        S("energy2")
        for mm in range(8):
            for j in range(4):
                n = 8 * j + mm
                L = lmax[n]
                if L <= 256:
                    continue
                nc.tensor.matmul(
                    ep[32 * j:32 * j + 8, 256:L], h2m[:, n * 8:(n + 1) * 8],
                    keys_sb[:, n * T + 256: n * T + L],
                    start=False, stop=(mm == e_stop2.get(j)),
                    tile_position=(0, 32 * j), skip_group_check=True)
        E()
        if maxL > 256:
            S("softmax2")
            nc.scalar.activation(exp_t[:, 256:512], ep[:, 256:512], AF.Exp)
            E()

        S("transp")
        for c in range(nch):
            nc.tensor.transpose(etp[:, c * 128:(c + 1) * 128],
                                exp_t[:, c * 128:(c + 1) * 128], ident_bf[:])
            src = bass.AP(etp[:].tensor, etp[:].offset + c * 128,
                          [etp[:].ap[0], [32, 4], [1, 8], [0, 8]])
            nc.vector.tensor_mul(
                attnTM[:, c * 256:(c + 1) * 256].rearrange(
                    "p (j mm m) -> p j mm m", j=4, mm=8),
                src,
                maskind_sb[:, c * 256:(c + 1) * 256].rearrange(
                    "p (j mm m) -> p j mm m", j=4, mm=8))
            if c == 1 or c == nch - 1:
                # context accumulation for the chunks transposed so far
                S("ctx")
                c_lo = 0 if c <= 1 else 2
                for cc in range(c_lo, c + 1):
                    for mm in range(8):
                        for j in range(4):
                            n = 8 * j + mm
                            if cc >= cmax[n]:
                                continue
                            nc.tensor.matmul(
                                cp[32 * j:32 * j + 8, 0:VSP],
                                attnTM[:, cc * 256 + n * 8:
                                       cc * 256 + (n + 1) * 8],
                                vals_sb[:, (cc * NB + n) * VSP:
                                        (cc * NB + n + 1) * VSP],
                                start=((cc, mm) == c_first[j]),
                                stop=((cc, mm) == c_last[j]),
                                tile_position=(0, 32 * j),
                                skip_group_check=True)
                E()
        E()

        S("ctxfin")
        nc.vector.reciprocal(rcp[:], cp[:, 128:129])
        ctxbm = ls_p.tile([128, 128], F16, tag="ctxbm")
        nc.vector.tensor_scalar_mul(ctxbm[:], cp[:, 0:128], rcp[:])
        filler(2)
        nc.tensor.transpose(etp[:, 512:640], ctxbm[:], ident_bf[:])
        nc.vector.tensor_copy(ctxb[:], etp[:, 512:640])
        E()

        if t + 1 < tl:
            emit_w1c(g1next)

        S("logits2")
        ctx_rhs2 = _scat_ap(ctxb)
        for mo in range(8):
            nc.tensor.matmul(lp[:, mo * 32:(mo + 1) * 32],
                             wout_sb[:, VOCP + mo * 128: VOCP + (mo + 1) * 128],
                             ctx_rhs2, start=False, stop=True,
                             skip_group_check=True)
        ls = ls_p.tile([128, 256], F16, tag="ls")
        bout_bc = bass.AP(bout_sb[:].tensor, bout_sb[:].offset,
                          [bout_sb[:].ap[0], [1, 8], [0, 32]])
        nc.vector.tensor_add(
            ls[:].rearrange("p (mo b) -> p mo b", mo=8),
            lp[:, 0:256].rearrange("p (mo b) -> p mo b", mo=8), bout_bc)
        dst = bass.AP(out_t, t * 8 * 128 * 32,
                      [[32, 128], [128 * 32, 8], [1, 32]])
        nc.gpsimd.dma_start(dst, ls[:].rearrange("p (mo b) -> p mo b", mo=8))
        E()

        if t + 1 < tl:
            g1cur = g1next
            xcur = xnext
            xnext = xnext2 if t + 2 < tl else None

    for p in reversed(pools):
        p.release()


@functools.lru_cache(maxsize=4)
def _build(tl, lmax, hb1, hb2):
    nc = bacc.Bacc("TRN2", target_bir_lowering=False, debug=False)
    io = {}
    io["keys"] = nc.dram_tensor("keys", [128, NB * T], F16, kind="ExternalInput")
    io["vals"] = nc.dram_tensor("vals", [128, NCH * NB * VSP], F16, kind="ExternalInput")
    io["xemb"] = nc.dram_tensor("xemb", [4, 128, TL * NB], F16, kind="ExternalInput")
    io["w1e"] = nc.dram_tensor("w1e", [128, 4 * G1], F16, kind="ExternalInput")
    io["whh1"] = nc.dram_tensor("whh1", [128, 4 * G1], F16, kind="ExternalInput")
    io["w1c"] = nc.dram_tensor("w1c", [128, G1], F16, kind="ExternalInput")
    io["w2i"] = nc.dram_tensor("w2i", [128, 4 * G2], F16, kind="ExternalInput")
    io["w2h"] = nc.dram_tensor("w2h", [128, G2], F16, kind="ExternalInput")
    io["wout"] = nc.dram_tensor("wout", [128, 2 * VOCP], F16, kind="ExternalInput")
    io["b1"] = nc.dram_tensor("b1", [128, 16], F32, kind="ExternalInput")
    io["b2"] = nc.dram_tensor("b2", [128, 4], F32, kind="ExternalInput")
    io["bout"] = nc.dram_tensor("bout", [128, 8], F32, kind="ExternalInput")
    io["ctx0"] = nc.dram_tensor("ctx0", [128, 128], F16, kind="ExternalInput")
    io["maskind"] = nc.dram_tensor("maskind", [128, NCH * 256], F16, kind="ExternalInput")
    io["out"] = nc.dram_tensor("out", [TL, 8, 128, 32], F16, kind="ExternalOutput")

    with tile.TileContext(nc) as tc:
        _emit(tc, io, tl, lmax, hb1, hb2)
    nc.compile()
    return nc


def _bf(x):
    return np.asarray(x, np.float32).astype(np.float16)


def _scat_perm():
    n = np.arange(NB)
    return 32 * (n // 8) + n % 8


def prep_inputs(key, values, lens, text, emb, W_ih1, W_hh1, b_ih1, b_hh1,
                W_ih2, W_hh2, b_ih2, b_hh2, W_out, b_out):
    key = np.asarray(key, np.float32)
    values = np.asarray(values, np.float32)
    lens = np.asarray(lens).astype(np.int64)
    text = np.asarray(text).astype(np.int64)
    emb = np.asarray(emb, np.float32)
    W_ih1 = np.asarray(W_ih1, np.float32)
    W_hh1 = np.asarray(W_hh1, np.float32)
    W_ih2 = np.asarray(W_ih2, np.float32)
    W_hh2 = np.asarray(W_hh2, np.float32)
    W_out = np.asarray(W_out, np.float32)
    b1 = np.asarray(b_ih1, np.float32) + np.asarray(b_hh1, np.float32)
    b2 = np.asarray(b_ih2, np.float32) + np.asarray(b_hh2, np.float32)
    b_out = np.asarray(b_out, np.float32)

    perm = _scat_perm()

    # sigmoid-via-tanh input scales (i, f, o rows) and the H = 2h / S = 2c
    # state-scaling compensation on consumer weights
    rs1 = np.ones((4 * H, 1), np.float32)
    rs1[0:2 * H] = 0.5          # i, f
    rs1[3 * H:4 * H] = 0.5      # o
    rs2 = np.ones((4 * KS, 1), np.float32)
    rs2[0:2 * KS] = 0.5
    rs2[3 * KS:4 * KS] = 0.5

    W1 = W_ih1 * rs1
    Wh1 = W_hh1 * rs1 * 0.5
    W2i = W_ih2 * rs2 * 0.5
    W2h = W_hh2 * rs2 * 0.5
    b1s = b1 * rs1.ravel()
    b2s = b2 * rs2.ravel()

    shared = {}
    w1T = np.ascontiguousarray(W1.T)  # (640, 2048)
    shared["w1e"] = _bf(w1T[:H].reshape(4, 128, G1).transpose(1, 0, 2).reshape(128, 4 * G1))
    shared["w1c"] = _bf(w1T[H:])
    shared["whh1"] = _bf(Wh1.T.reshape(4, 128, G1).transpose(1, 0, 2).reshape(128, 4 * G1))
    shared["w2i"] = _bf(W2i.T.reshape(4, 128, G2).transpose(1, 0, 2).reshape(128, 4 * G2))
    shared["w2h"] = _bf(np.ascontiguousarray(W2h.T))
    woutp = np.zeros((VOCP, KS + VS), np.float32)
    woutp[:VOC] = W_out
    woutp[:, :KS] *= 0.5        # h2 = H2/2
    shared["wout"] = _bf(woutp.T.reshape(2, 128, VOCP).transpose(1, 0, 2).reshape(128, 2 * VOCP))
    shared["b1"] = np.ascontiguousarray(b1s.reshape(16, 128).T)
    shared["b2"] = np.ascontiguousarray(b2s.reshape(4, 128).T)
    boutp = np.zeros((VOCP,), np.float32)
    boutp[:VOC] = b_out
    shared["bout"] = np.ascontiguousarray(boutp.reshape(8, 128).T)

    # sort batches into slots by length (ascending) per core; the slot-wise
    # max over cores defines the compile-time length profile
    lens_c = lens.reshape(NCORES, NB)
    orders = [np.argsort(lens_c[c], kind="stable") for c in range(NCORES)]
    slot_lens = np.stack([lens_c[c][orders[c]] for c in range(NCORES)])
    lmax = tuple(int(v) for v in slot_lens.max(axis=0))

    in_maps = []
    for core in range(NCORES):
        sl = slice(core * NB, (core + 1) * NB)
        order = orders[core]
        keyc = key[:, sl, :][:, order, :]
        valc = values[:, sl, :][:, order, :]
        lensc = lens[sl][order]
        textc = text[sl][order]

        m = dict(shared)
        # zero the invalid (t >= len) key rows (masked energies become 0)
        # and fold the H2 = 2*h2 compensation into the keys
        kz = keyc * 0.5 * (np.arange(T)[:, None, None] < lensc[None, :, None])
        m["keys"] = _bf(np.ascontiguousarray(
            kz.transpose(2, 1, 0)).reshape(128, NB * T))
        vp = np.zeros((NCH * 128, NB, VSP), np.float32)
        vp[:T, :, :VS] = valc
        vp[:, :, VS] = 1.0       # ones column -> softmax denominator
        m["vals"] = _bf(np.ascontiguousarray(
            vp.reshape(NCH, 128, NB * VSP).transpose(1, 0, 2)).reshape(
                128, NCH * NB * VSP))
        embs = emb[textc]                       # (32, TL, H)
        m["xemb"] = _bf(np.ascontiguousarray(
            embs.transpose(2, 1, 0)).reshape(4, 128, TL * NB))
        ctx0 = valc.mean(axis=0)                # (32, VS)
        c0 = np.zeros((128, 128), np.float32)
        c0[:, perm] = ctx0.T
        m["ctx0"] = _bf(c0)
        # maskind[t', c*256 + n*8 + m] = (m==n%8) * (c*128+t' < len_n)
        ind = (np.arange(NCH * 128)[None, :] < lensc[:, None]).astype(np.float32)
        mi = np.zeros((128, NCH, 32, 8), np.float32)
        nn = np.arange(NB)
        mi[:, :, nn, nn % 8] = ind.reshape(NB, NCH, 128).transpose(2, 1, 0)
        m["maskind"] = _bf(mi.reshape(128, NCH * 256))
        in_maps.append(m)
    return in_maps, orders, lmax, b1s, b2s


def kernel(key, values, lens, text, emb, W_ih1, W_hh1, b_ih1, b_hh1,
           W_ih2, W_hh2, b_ih2, b_hh2, W_out, b_out,
           _trace=False, _tl=TL):
    in_maps, orders, lmax, b1s, b2s = prep_inputs(
        key, values, lens, text, emb, W_ih1, W_hh1, b_ih1, b_hh1,
        W_ih2, W_hh2, b_ih2, b_hh2, W_out, b_out)
    hb1 = bool(np.any(b1s))
    hb2 = bool(np.any(b2s))
    if __import__('os').environ.get('NOTRIM'):
        lmax = tuple(T for _ in lmax)
    nc = _build(_tl, lmax, hb1, hb2)
    res = bass_utils.run_bass_kernel_spmd(
        nc, in_maps, core_ids=list(range(NCORES)), trace=_trace)
    kernel._last_results = res

    full = np.zeros((NCORES * NB, TL, VOC), np.float32)
    for core in range(NCORES):
        o = np.asarray(res.results[core]["out"]).astype(np.float32)
        o = o.reshape(TL, VOCP, 32)
        full[core * NB + orders[core]] = o[:, :VOC, :].transpose(2, 0, 1)
    return full


# revision 30
# speedup vs baseline: 1.1008x; 1.0642x over previous
"""Trainium2 Bass kernel for the attention-LSTM decoder (LAS-style).

Sharding: data-parallel over batch N=256 -> 32 per core across 8 cores.
Per-core layout is feature-major (features on SBUF partitions, batch in the
free dimension).  The 250-step recurrence runs fully unrolled on-device.

Structure (v2 of the kernel; rebuilt around trace measurements showing the
PE is weight-load bound and ~25% idle during serial ACT/DVE windows):
  - Activations via the Tanh table only (sigmoid(x) = 0.5 + 0.5*tanh(x/2),
    input scales folded into host-side weights, output affines fused into
    DVE scalar_tensor_tensor ops).  State is stored as H = 2h / S = 2c.
    exp/tanh live in one ACT table -> no table reloads.
  - Attention uses single-column stationaries: for sample n the energy
    matmul is keys[:, n]^T @ h2[:, n:n+1] written to ONE psum partition
    (scat(n) = 32*(n//8) + n%8, spreading samples across the four PE
    column-tiles).  No one-hot masks, no h2m staging; chunks of the
    encoder axis are merged into one long-stream matmul per sample with
    compile-time exact lengths (slot-sorted by length on the host).
  - Same trick for context: per (sample, chunk) a [128,1] stationary of
    masked-exp attn weights against the values block; the ones-column of
    values yields the softmax denominator.
  - Scheduling: W1e@x(t+1) runs under tanh/DVE of gates1; Whh1@h1 runs
    under the lstm2 tanh window; the logits h2-half runs under exp; the
    energy/exp/transpose/mask/ctx pipeline is split at t=256 so ACT and
    PE overlap; logits bias-add + output DMA live on the Pool engine.
"""

import functools
import sys

for _p in ("/opt/trn_rl_repo",):
    if _p not in sys.path:
        sys.path.insert(0, _p)

import numpy as np
import ml_dtypes

import concourse.bass as bass
import concourse.tile as tile
from concourse import bacc, mybir
from concourse import bass_utils
from concourse.masks import make_identity

dt = mybir.dt
AF = mybir.ActivationFunctionType
ALU = mybir.AluOpType

NCORES = 8
NB = 32            # batch per core
T = 400            # encoder length
NCH = 4            # time chunks of 128
KS = 128
VS = 128
VSP = VS + 1       # values + ones column (softmax denominator)
H = 512
G1 = 2048          # 4*H
G2 = 512           # 4*KS
TL = 250
VOC = 1000
VOCP = 1024

F16 = dt.float16
F32 = dt.float32
F8 = dt.float8e4
WSC = 64.0  # fp8 weight scale (folded out via ACT scale / logits STT)


def _scat(n):
    return 32 * (n // 8) + n % 8


def _scat_ap(t):
    """AP over the 32 'scattered' columns {32j+m : j<4, m<8} of a 128-col
    feature-major tile, ordered n = 8j+m."""
    a = t[:]
    return bass.AP(a.tensor, a.offset, [a.ap[0], [32, 4], [1, 8]])


def _emit(tc, io, tl, lmax, hb1, hb2):
    nc = tc.nc
    pools = []
    _scopes = []

    def S(name):
        _scopes.append((name, nc.enter_named_scope(name, False)[0]))

    def E():
        n, i = _scopes.pop()
        nc.leave_named_scope(n, i, False)

    def pool(name, bufs, space="SBUF"):
        p = tc.alloc_tile_pool(name=name, bufs=bufs, space=space)
        pools.append(p)
        return p

    maxL = max(lmax)
    nch = (maxL + 127) // 128

    const = pool("const", 1)
    state = pool("state", 1)
    gate_p = pool("gate", 2)
    ls_p = pool("ls", 2)
    xet_p = pool("xet", 3)
    g1_p = pool("g1", 2, space="PSUM")
    pstate = pool("pstate", 1, space="PSUM")

    # ---- resident constants ----
    def load_const(key, shape, dtype):
        t = const.tile(shape, dtype, name=key + "_sb")
        nc.sync.dma_start(t[:], io[key].ap())
        return t

    w1e_sb = load_const("w1e", [128, 4 * G1], F8)
    keys_sb = load_const("keys", [128, NB * T], F16)
    vals_sb = load_const("vals", [128, NCH * NB * VSP], F16)
    whh1_sb = load_const("whh1", [128, 4 * G1], F8)
    w1c_sb = load_const("w1c", [128, G1], F8)
    w2i_sb = load_const("w2i", [128, 4 * G2], F8)
    w2h_sb = load_const("w2h", [128, G2], F8)
    wout_sb = load_const("wout", [128, 2 * VOCP], F16)
    bout_sb = load_const("bout", [128, 8], F32)
    # maskind[t', c*256 + n*8 + m] = (m==n%8) * (c*128+t' < len_n)
    maskind_sb = load_const("maskind", [128, NCH * 256], F16)
    if hb1:
        b1_sb = load_const("b1", [128, 16], F32)
    if hb2:
        b2_sb = load_const("b2", [128, 4], F32)

    ident_bf = const.tile([128, 128], F16, name="ident_bf")
    make_identity(nc, ident_bf[:])
    # mask8[p, n*8+m] = 1.0 iff m == n%8 (same on every partition)
    mask8 = const.tile([128, 256], F32, name="mask8")
    nc.gpsimd.memset(mask8[:], 0.0)
    nc.gpsimd.affine_select(
        out=mask8[:], in_=mask8[:],
        compare_op=ALU.not_equal, fill=1.0, base=0,
        pattern=[[0, 4], [-1, 8], [1, 8]], channel_multiplier=0,
    )
    mask8_32x8 = bass.AP(mask8[:].tensor, mask8[:].offset,
                         [mask8[:].ap[0], [8, 32], [1, 8]])

    # ---- state ----
    # gw: [i f g o | s1] (tnh output cols 0:512, S1 = 2*c1 cols 512:640)
    gw = state.tile([128, 640], F32, name="gw")
    gw2 = state.tile([128, 160], F32, name="gw2")   # [i f g o | s2], 32 each
    h1b = state.tile([128, 128], F16, name="h1b")   # H1 = 2*h1
    h2b = state.tile([128, 32], F16, name="h2b")    # H2 = 2*h2
    ctxb = state.tile([128, 128], F16, name="ctxb")  # scattered cols
    exp_t = state.tile([128, 512], F16, name="exp_t")
    attnTM = state.tile([128, NCH * 256], F16, name="attnTM")
    h2m = state.tile([128, 256], F16, name="h2m")
    rcp = state.tile([128, 1], F32, name="rcp")

    for tl_ in (gw, gw2):
        nc.vector.memset(tl_[:], 0.0)
    for tl_ in (h1b, h2b, exp_t):
        nc.vector.memset(tl_[:], 0.0)
    nc.sync.dma_start(ctxb[:], io["ctx0"].ap())

    # persistent psum tiles (1:1 bank reuse across steps; stale regions are
    # finite and get masked after exp)
    ep = pstate.tile([128, 512], F32, name="ep")
    cp = pstate.tile([128, 512], F32, name="cp")
    lp_t = pstate.tile([128, 512], F32, name="lp")
    g2p_t = pstate.tile([128, 512], F32, name="g2p")
    etp = pstate.tile([128, 1024], F16, name="etp")  # expT 0:512 | ctxT 512:640
    for tl_ in (ep, cp):
        nc.vector.memset(tl_[:], 0.0)
    # denominator col reads 1.0 on the 96 dead partitions (finite rcp)
    nc.vector.memset(cp[:, 128:129], 1.0)
    lp = lp_t[:, 0:256]
    g2p = g2p_t[:, 0:128]

    out_t = io["out"].ap().tensor     # [TL, 8, 128, 32] f16
    xemb_t = io["xemb"].ap().tensor   # [4, 128, TL*NB] f16

    def emit_xet_dma(t):
        xt = xet_p.tile([128, 4 * 32], F16, tag="xet")
        src = bass.AP(xemb_t, t * NB,
                      [[TL * NB, 128], [128 * TL * NB, 4], [1, 32]])
        nc.sync.dma_start(xt[:].rearrange("p (k b) -> p k b", k=4), src)
        return xt

    W1E_SPLIT = 6

    def emit_w1e(g1t, xt, m_lo=0, m_hi=16):
        # g1 partial: W1e @ xemb(t).  ONE start for the whole bank: the
        # pend covers all 2KB, each region's first write resets it, later
        # Whh1/W1c matmuls accumulate.
        S("w1e")
        for m in range(m_lo, m_hi):
            o = m * 32
            for k in range(4):
                nc.tensor.matmul(
                    g1t[:, o:o + 32],
                    w1e_sb[:, k * G1 + m * 128: k * G1 + (m + 1) * 128],
                    xt[:, k * 32:(k + 1) * 32],
                    start=(m == 0 and k == 0), stop=False,
                    skip_group_check=True)
        E()

    def emit_whh1(g1t):
        S("whh1")
        for m in range(16):
            o = m * 32
            for k in range(4):
                nc.tensor.matmul(
                    g1t[:, o:o + 32],
                    whh1_sb[:, k * G1 + m * 128: k * G1 + (m + 1) * 128],
                    h1b[:, k * 32:(k + 1) * 32],
                    start=False, stop=False, skip_group_check=True)
        E()

    def emit_w1c(g1t):
        S("w1c")
        ctx_rhs = _scat_ap(ctxb)
        for m in range(16):
            o = m * 32
            nc.tensor.matmul(
                g1t[:, o:o + 32], w1c_sb[:, m * 128:(m + 1) * 128],
                ctx_rhs, start=False, stop=(m == 15), skip_group_check=True)
        E()

    def pair_ap(tile_, off, stride, w):
        a = tile_[:]
        return bass.AP(a.tensor, a.offset + off, [a.ap[0], [stride, 2], [1, w]])

    # per-quadrant chain bounds: energy half-1 stop (if no half-2 in the
    # quadrant), half-2 stop; ctx first/last (c, mm) pairs
    e_stop1 = {}
    e_stop2 = {}
    for j in range(4):
        long_mm = [mm for mm in range(8) if lmax[8 * j + mm] > 256]
        e_stop1[j] = 7 if not long_mm else None
        if long_mm:
            e_stop2[j] = long_mm[-1]
    cmax = [(lmax[n] + 127) // 128 for n in range(NB)]
    c_pairs = {j: [(c, mm) for c in range(nch) for mm in range(8)
                   if c < cmax[8 * j + mm]] for j in range(4)}
    c_first = {j: p[0] for j, p in c_pairs.items()}
    c_last = {j: p[-1] for j, p in c_pairs.items()}

    # ---- prologue ----
    xcur = emit_xet_dma(0)
    xnext = emit_xet_dma(1) if tl > 1 else None
    g1cur = g1_p.tile([128, 512], F32, tag="g1")
    emit_w1e(g1cur, xcur)
    emit_whh1(g1cur)
    emit_w1c(g1cur)
    g1next = None
    if tl > 1:
        g1next = g1_p.tile([128, 512], F32, tag="g1")
        emit_w1e(g1next, xnext, 0, W1E_SPLIT)

    # ---- the recurrence ----
    for t in range(tl):
        if t + 2 < tl:
            xnext2 = emit_xet_dma(t + 2)

        # PE: rest of next step's W1e chain fills the gates1 ACT/DVE window
        # (the first W1E_SPLIT m-regions ran in the previous ctxfin window)
        if t + 1 < tl:
            emit_w1e(g1next, xnext, W1E_SPLIT, 16)

        S("gates1")
        if hb1:
            g1sb = gate_p.tile([128, 512], F32, tag="g1sb")
            b1_bc = bass.AP(b1_sb[:].tensor, b1_sb[:].offset,
                            [b1_sb[:].ap[0], [1, 16], [0, 32]])
            nc.vector.tensor_add(
                g1sb[:].rearrange("p (m b) -> p m b", m=16),
                g1cur[:].rearrange("p (m b) -> p m b", m=16), b1_bc)
            nc.scalar.activation(gw[:, 0:512], g1sb[:], AF.Tanh, scale=1.0 / WSC)
        else:
            nc.scalar.activation(gw[:, 0:512], g1cur[:], AF.Tanh, scale=1.0 / WSC)
        # a2 = (i+1)*g ; a1 = (f+1)*S1 ; S1 = a1*0.5 + a2
        aa = gate_p.tile([128, 256], F32, tag="aa")
        nc.vector.scalar_tensor_tensor(
            aa[:, 0:128], gw[:, 0:128], 1.0, gw[:, 256:384],
            ALU.add, ALU.mult)
        nc.vector.scalar_tensor_tensor(
            aa[:, 128:256], gw[:, 128:256], 1.0, gw[:, 512:640],
            ALU.add, ALU.mult)
        nc.vector.scalar_tensor_tensor(
            gw[:, 512:640], aa[:, 128:256], 0.5, aa[:, 0:128],
            ALU.mult, ALU.add)
        th = gate_p.tile([128, 128], F32, tag="th")
        nc.scalar.activation(th[:], gw[:, 512:640], AF.Tanh, scale=0.5)
        nc.vector.scalar_tensor_tensor(
            h1b[:], gw[:, 384:512], 1.0, th[:], ALU.add, ALU.mult)
        E()

        S("lstm2")
        for m in range(4):
            o = m * 32
            nc.tensor.matmul(
                g2p[:, o:o + 32], w2h_sb[:, m * 128:(m + 1) * 128],
                h2b[:], start=True, stop=False, skip_group_check=True)
            for k in range(4):
                nc.tensor.matmul(
                    g2p[:, o:o + 32],
                    w2i_sb[:, k * G2 + m * 128: k * G2 + (m + 1) * 128],
                    h1b[:, k * 32:(k + 1) * 32],
                    start=False, stop=(k == 3), skip_group_check=True)
        if hb2:
            g2sb = gate_p.tile([128, 128], F32, tag="g2sb")
            b2_bc = bass.AP(b2_sb[:].tensor, b2_sb[:].offset,
                            [b2_sb[:].ap[0], [1, 4], [0, 32]])
            nc.vector.tensor_add(
                g2sb[:].rearrange("p (m b) -> p m b", m=4),
                g2p[:].rearrange("p (m b) -> p m b", m=4), b2_bc)
            nc.scalar.activation(gw2[:, 0:128], g2sb[:], AF.Tanh, scale=1.0 / WSC)
        else:
            nc.scalar.activation(gw2[:, 0:128], g2p[:], AF.Tanh, scale=1.0 / WSC)
        aa2 = gate_p.tile([128, 64], F32, tag="aa2")
        nc.vector.scalar_tensor_tensor(
            aa2[:, 0:32], gw2[:, 0:32], 1.0, gw2[:, 64:96],
            ALU.add, ALU.mult)
        nc.vector.scalar_tensor_tensor(
            aa2[:, 32:64], gw2[:, 32:64], 1.0, gw2[:, 128:160],
            ALU.add, ALU.mult)
        nc.vector.scalar_tensor_tensor(
            gw2[:, 128:160], aa2[:, 32:64], 0.5, aa2[:, 0:32],
            ALU.mult, ALU.add)
        th2 = gate_p.tile([128, 32], F32, tag="th2")
        nc.scalar.activation(th2[:], gw2[:, 128:160], AF.Tanh, scale=0.5)
        nc.vector.scalar_tensor_tensor(
            h2b[:], gw2[:, 96:128], 1.0, th2[:], ALU.add, ALU.mult)
        h2bc = bass.AP(h2b[:].tensor, h2b[:].offset,
                       [h2b[:].ap[0], [1, 32], [0, 8]])
        nc.vector.tensor_mul(
            h2m[:].rearrange("p (n m) -> p n m", m=8), h2bc, mask8_32x8)
        E()

        # PE: Whh1 @ h1(t) for step t+1 fills the lstm2 tanh/DVE window
        if t + 1 < tl:
            emit_whh1(g1next)

        S("energy")
        # one long-stream matmul per (sample, half) with one-hot-masked h2
        # columns writing that sample's psum row; the encoder axis is split
        # at col 256 so exp of the first half overlaps the tail.  Chains
        # accumulate per quadrant (column tile).
        for mm in range(8):
            for j in range(4):
                n = 8 * j + mm
                L = lmax[n]
                L1 = min(L, 256)
                nc.tensor.matmul(
                    ep[32 * j:32 * j + 8, 0:L1], h2m[:, n * 8:(n + 1) * 8],
                    keys_sb[:, n * T: n * T + L1],
                    start=(mm == 0), stop=(mm == e_stop1[j]),
                    tile_position=(0, 32 * j), skip_group_check=True)
        E()

        S("logits")
        # h2 half of the logits: independent PE work under the exp window.
        # ONE start for the whole bank -- a per-region start would re-pend
        # the earlier regions' bytes and the ctx-half accumulation below
        # would then overwrite instead of accumulate.
        for mo in range(8):
            nc.tensor.matmul(lp[:, mo * 32:(mo + 1) * 32],
                             wout_sb[:, mo * 128:(mo + 1) * 128],
                             h2b[:], start=(mo == 0), stop=False,
                             skip_group_check=True)
        E()

        S("softmax")
        nc.scalar.activation(exp_t[:, 0:256], ep[:, 0:256], AF.Exp)
        E()

        S("energy2")
        for mm in range(8):
            for j in range(4):
                n = 8 * j + mm
                L = lmax[n]
                if L <= 256:
                    continue
                nc.tensor.matmul(
                    ep[32 * j:32 * j + 8, 256:L], h2m[:, n * 8:(n + 1) * 8],
                    keys_sb[:, n * T + 256: n * T + L],
                    start=False, stop=(mm == e_stop2.get(j)),
                    tile_position=(0, 32 * j), skip_group_check=True)
        E()
        if maxL > 256:
            S("softmax2")
            nc.scalar.activation(exp_t[:, 256:512], ep[:, 256:512], AF.Exp)
            E()

        S("transp")
        for c in range(nch):
            nc.tensor.transpose(etp[:, c * 128:(c + 1) * 128],
                                exp_t[:, c * 128:(c + 1) * 128], ident_bf[:])
            src = bass.AP(etp[:].tensor, etp[:].offset + c * 128,
                          [etp[:].ap[0], [32, 4], [0, 8], [1, 8]])
            nc.vector.tensor_mul(
                attnTM[:, c * 256:(c + 1) * 256].rearrange(
                    "p (j mm m) -> p j mm m", j=4, mm=8),
                src,
                maskind_sb[:, c * 256:(c + 1) * 256].rearrange(
                    "p (j mm m) -> p j mm m", j=4, mm=8))
            if c == 1 or c == nch - 1:
                # context accumulation for the chunks transposed so far
                S("ctx")
                c_lo = 0 if c <= 1 else 2
                for cc in range(c_lo, c + 1):
                    for mm in range(8):
                        for j in range(4):
                            n = 8 * j + mm
                            if cc >= cmax[n]:
                                continue
                            nc.tensor.matmul(
                                cp[32 * j:32 * j + 8, 0:VSP],
                                attnTM[:, cc * 256 + n * 8:
                                       cc * 256 + (n + 1) * 8],
                                vals_sb[:, (cc * NB + n) * VSP:
                                        (cc * NB + n + 1) * VSP],
                                start=((cc, mm) == c_first[j]),
                                stop=((cc, mm) == c_last[j]),
                                tile_position=(0, 32 * j),
                                skip_group_check=True)
                E()
        E()

        S("ctxfin")
        nc.vector.reciprocal(rcp[:], cp[:, 128:129])
        ctxbm = ls_p.tile([128, 128], F16, tag="ctxbm")
        nc.vector.tensor_scalar_mul(ctxbm[:], cp[:, 0:128], rcp[:])
        # W1e part-a for step t+2 fills the rcp/ctxbm window on the PE
        g1nn = None
        if t + 2 < tl:
            g1nn = g1_p.tile([128, 512], F32, tag="g1")
            emit_w1e(g1nn, xnext2, 0, W1E_SPLIT)
        nc.tensor.transpose(etp[:, 512:640], ctxbm[:], ident_bf[:])
        nc.vector.tensor_copy(ctxb[:], etp[:, 512:640])
        E()

        if t + 1 < tl:
            emit_w1c(g1next)

        S("logits2")
        ctx_rhs2 = _scat_ap(ctxb)
        for mo in range(8):
            nc.tensor.matmul(lp[:, mo * 32:(mo + 1) * 32],
                             wout_sb[:, VOCP + mo * 128: VOCP + (mo + 1) * 128],
                             ctx_rhs2, start=False, stop=True,
                             skip_group_check=True)
        ls = ls_p.tile([128, 256], F16, tag="ls")
        bout_bc = bass.AP(bout_sb[:].tensor, bout_sb[:].offset,
                          [bout_sb[:].ap[0], [1, 8], [0, 32]])
        nc.vector.tensor_add(
            ls[:].rearrange("p (mo b) -> p mo b", mo=8),
            lp[:, 0:256].rearrange("p (mo b) -> p mo b", mo=8), bout_bc)
        dst = bass.AP(out_t, t * 8 * 128 * 32,
                      [[32, 128], [128 * 32, 8], [1, 32]])
        nc.gpsimd.dma_start(dst, ls[:].rearrange("p (mo b) -> p mo b", mo=8))
        E()

        if t + 1 < tl:
            g1cur = g1next
            g1next = g1nn
            xcur = xnext
            xnext = xnext2 if t + 2 < tl else None

    for p in reversed(pools):
        p.release()


@functools.lru_cache(maxsize=4)
def _build(tl, lmax, hb1, hb2):
    nc = bacc.Bacc("TRN2", target_bir_lowering=False, debug=False)
    io = {}
    io["keys"] = nc.dram_tensor("keys", [128, NB * T], F16, kind="ExternalInput")
    io["vals"] = nc.dram_tensor("vals", [128, NCH * NB * VSP], F16, kind="ExternalInput")
    io["xemb"] = nc.dram_tensor("xemb", [4, 128, TL * NB], F16, kind="ExternalInput")
    io["w1e"] = nc.dram_tensor("w1e", [128, 4 * G1], F8, kind="ExternalInput")
    io["whh1"] = nc.dram_tensor("whh1", [128, 4 * G1], F8, kind="ExternalInput")
    io["w1c"] = nc.dram_tensor("w1c", [128, G1], F8, kind="ExternalInput")
    io["w2i"] = nc.dram_tensor("w2i", [128, 4 * G2], F8, kind="ExternalInput")
    io["w2h"] = nc.dram_tensor("w2h", [128, G2], F8, kind="ExternalInput")
    io["wout"] = nc.dram_tensor("wout", [128, 2 * VOCP], F16, kind="ExternalInput")
    io["b1"] = nc.dram_tensor("b1", [128, 16], F32, kind="ExternalInput")
    io["b2"] = nc.dram_tensor("b2", [128, 4], F32, kind="ExternalInput")
    io["bout"] = nc.dram_tensor("bout", [128, 8], F32, kind="ExternalInput")
    io["ctx0"] = nc.dram_tensor("ctx0", [128, 128], F16, kind="ExternalInput")
    io["maskind"] = nc.dram_tensor("maskind", [128, NCH * 256], F16, kind="ExternalInput")
    io["out"] = nc.dram_tensor("out", [TL, 8, 128, 32], F16, kind="ExternalOutput")

    with tile.TileContext(nc) as tc:
        _emit(tc, io, tl, lmax, hb1, hb2)
    nc.compile()
    return nc


def _bf(x):
    return np.asarray(x, np.float32).astype(np.float16)


def _f8(x):
    return (np.asarray(x, np.float32) * WSC).astype(ml_dtypes.float8_e4m3)


def _scat_perm():
    n = np.arange(NB)
    return 32 * (n // 8) + n % 8


def prep_inputs(key, values, lens, text, emb, W_ih1, W_hh1, b_ih1, b_hh1,
                W_ih2, W_hh2, b_ih2, b_hh2, W_out, b_out):
    key = np.asarray(key, np.float32)
    values = np.asarray(values, np.float32)
    lens = np.asarray(lens).astype(np.int64)
    text = np.asarray(text).astype(np.int64)
    emb = np.asarray(emb, np.float32)
    W_ih1 = np.asarray(W_ih1, np.float32)
    W_hh1 = np.asarray(W_hh1, np.float32)
    W_ih2 = np.asarray(W_ih2, np.float32)
    W_hh2 = np.asarray(W_hh2, np.float32)
    W_out = np.asarray(W_out, np.float32)
    b1 = np.asarray(b_ih1, np.float32) + np.asarray(b_hh1, np.float32)
    b2 = np.asarray(b_ih2, np.float32) + np.asarray(b_hh2, np.float32)
    b_out = np.asarray(b_out, np.float32)

    perm = _scat_perm()

    # sigmoid-via-tanh input scales (i, f, o rows) and the H = 2h / S = 2c
    # state-scaling compensation on consumer weights
    rs1 = np.ones((4 * H, 1), np.float32)
    rs1[0:2 * H] = 0.5          # i, f
    rs1[3 * H:4 * H] = 0.5      # o
    rs2 = np.ones((4 * KS, 1), np.float32)
    rs2[0:2 * KS] = 0.5
    rs2[3 * KS:4 * KS] = 0.5

    W1 = W_ih1 * rs1
    Wh1 = W_hh1 * rs1 * 0.5
    W2i = W_ih2 * rs2 * 0.5
    W2h = W_hh2 * rs2 * 0.5
    b1s = b1 * rs1.ravel()
    b2s = b2 * rs2.ravel()

    shared = {}
    w1T = np.ascontiguousarray(W1.T)  # (640, 2048)
    shared["w1e"] = _f8(w1T[:H].reshape(4, 128, G1).transpose(1, 0, 2).reshape(128, 4 * G1))
    shared["w1c"] = _f8(w1T[H:])
    shared["whh1"] = _f8(Wh1.T.reshape(4, 128, G1).transpose(1, 0, 2).reshape(128, 4 * G1))
    shared["w2i"] = _f8(W2i.T.reshape(4, 128, G2).transpose(1, 0, 2).reshape(128, 4 * G2))
    shared["w2h"] = _f8(np.ascontiguousarray(W2h.T))
    woutp = np.zeros((VOCP, KS + VS), np.float32)
    woutp[:VOC] = W_out
    woutp[:, :KS] *= 0.5        # h2 = H2/2
    shared["wout"] = _bf(woutp.T.reshape(2, 128, VOCP).transpose(1, 0, 2).reshape(128, 2 * VOCP))
    shared["b1"] = np.ascontiguousarray((b1s * WSC).reshape(16, 128).T)
    shared["b2"] = np.ascontiguousarray((b2s * WSC).reshape(4, 128).T)
    boutp = np.zeros((VOCP,), np.float32)
    boutp[:VOC] = b_out
    shared["bout"] = np.ascontiguousarray(boutp.reshape(8, 128).T)

    # sort batches into slots by length (ascending) per core; the slot-wise
    # max over cores defines the compile-time length profile
    lens_c = lens.reshape(NCORES, NB)
    orders = [np.argsort(lens_c[c], kind="stable") for c in range(NCORES)]
    slot_lens = np.stack([lens_c[c][orders[c]] for c in range(NCORES)])
    lmax = tuple(int(v) for v in slot_lens.max(axis=0))

    in_maps = []
    for core in range(NCORES):
        sl = slice(core * NB, (core + 1) * NB)
        order = orders[core]
        keyc = key[:, sl, :][:, order, :]
        valc = values[:, sl, :][:, order, :]
        lensc = lens[sl][order]
        textc = text[sl][order]

        m = dict(shared)
        # zero the invalid (t >= len) key rows (masked energies become 0)
        # and fold the H2 = 2*h2 compensation into the keys
        kz = keyc * 0.5 * (np.arange(T)[:, None, None] < lensc[None, :, None])
        m["keys"] = _bf(np.ascontiguousarray(
            kz.transpose(2, 1, 0)).reshape(128, NB * T))
        vp = np.zeros((NCH * 128, NB, VSP), np.float32)
        vp[:T, :, :VS] = valc
        vp[:, :, VS] = 1.0       # ones column -> softmax denominator
        m["vals"] = _bf(np.ascontiguousarray(
            vp.reshape(NCH, 128, NB * VSP).transpose(1, 0, 2)).reshape(
                128, NCH * NB * VSP))
        embs = emb[textc]                       # (32, TL, H)
        m["xemb"] = _bf(np.ascontiguousarray(
            embs.transpose(2, 1, 0)).reshape(4, 128, TL * NB))
        ctx0 = valc.mean(axis=0)                # (32, VS)
        c0 = np.zeros((128, 128), np.float32)
        c0[:, perm] = ctx0.T
        m["ctx0"] = _bf(c0)
        # maskind[t', c*256 + n*8 + m] = (m==n%8) * (c*128+t' < len_n)
        ind = (np.arange(NCH * 128)[None, :] < lensc[:, None]).astype(np.float32)
        mi = np.zeros((128, NCH, 32, 8), np.float32)
        nn = np.arange(NB)
        mi[:, :, nn, nn % 8] = ind.reshape(NB, NCH, 128).transpose(2, 1, 0)
        m["maskind"] = _bf(mi.reshape(128, NCH * 256))
        in_maps.append(m)
    return in_maps, orders, lmax, b1s, b2s


def kernel(key, values, lens, text, emb, W_ih1, W_hh1, b_ih1, b_hh1,
           W_ih2, W_hh2, b_ih2, b_hh2, W_out, b_out,
           _trace=False, _tl=TL):
    in_maps, orders, lmax, b1s, b2s = prep_inputs(
        key, values, lens, text, emb, W_ih1, W_hh1, b_ih1, b_hh1,
        W_ih2, W_hh2, b_ih2, b_hh2, W_out, b_out)
    hb1 = bool(np.any(b1s))
    hb2 = bool(np.any(b2s))
    if __import__('os').environ.get('NOTRIM'):
        lmax = tuple(T for _ in lmax)
    nc = _build(_tl, lmax, hb1, hb2)
    res = bass_utils.run_bass_kernel_spmd(
        nc, in_maps, core_ids=list(range(NCORES)), trace=_trace)
    kernel._last_results = res

    full = np.zeros((NCORES * NB, TL, VOC), np.float32)
    for core in range(NCORES):
        o = np.asarray(res.results[core]["out"]).astype(np.float32)
        o = o.reshape(TL, VOCP, 32)
        full[core * NB + orders[core]] = o[:, :VOC, :].transpose(2, 0, 1)
    return full


# revision 33
# speedup vs baseline: 1.1519x; 1.0464x over previous
"""Trainium2 Bass kernel for the attention-LSTM decoder (LAS-style).

Sharding: data-parallel over batch N=256 -> 32 per core across 8 cores.
Per-core layout is feature-major (features on SBUF partitions, batch in the
free dimension).  The 250-step recurrence runs fully unrolled on-device.

Structure (v2 of the kernel; rebuilt around trace measurements showing the
PE is weight-load bound and ~25% idle during serial ACT/DVE windows):
  - Activations via the Tanh table only (sigmoid(x) = 0.5 + 0.5*tanh(x/2),
    input scales folded into host-side weights, output affines fused into
    DVE scalar_tensor_tensor ops).  State is stored as H = 2h / S = 2c.
    exp/tanh live in one ACT table -> no table reloads.
  - Attention uses single-column stationaries: for sample n the energy
    matmul is keys[:, n]^T @ h2[:, n:n+1] written to ONE psum partition
    (scat(n) = 32*(n//8) + n%8, spreading samples across the four PE
    column-tiles).  No one-hot masks, no h2m staging; chunks of the
    encoder axis are merged into one long-stream matmul per sample with
    compile-time exact lengths (slot-sorted by length on the host).
  - Same trick for context: per (sample, chunk) a [128,1] stationary of
    masked-exp attn weights against the values block; the ones-column of
    values yields the softmax denominator.
  - Scheduling: W1e@x(t+1) runs under tanh/DVE of gates1; Whh1@h1 runs
    under the lstm2 tanh window; the logits h2-half runs under exp; the
    energy/exp/transpose/mask/ctx pipeline is split at t=256 so ACT and
    PE overlap; logits bias-add + output DMA live on the Pool engine.
"""

import functools
import sys

for _p in ("/opt/trn_rl_repo",):
    if _p not in sys.path:
        sys.path.insert(0, _p)

import numpy as np
import ml_dtypes

import concourse.bass as bass
import concourse.tile as tile
from concourse import bacc, mybir
from concourse import bass_utils
from concourse.masks import make_identity

dt = mybir.dt
AF = mybir.ActivationFunctionType
ALU = mybir.AluOpType

NCORES = 8
NB = 32            # batch per core
T = 400            # encoder length
NCH = 4            # time chunks of 128
KS = 128
VS = 128
VSP = VS + 1       # values + ones column (softmax denominator)
H = 512
G1 = 2048          # 4*H
G2 = 512           # 4*KS
TL = 250
VOC = 1000
VOCP = 1024

F16 = dt.float16
F32 = dt.float32
F8 = dt.float8e4
WSC = 64.0  # fp8 weight scale (folded out via ACT scale / logits STT)


def _scat(n):
    return 32 * (n // 8) + n % 8


def _scat_ap(t):
    """AP over the 32 'scattered' columns {32j+m : j<4, m<8} of a 128-col
    feature-major tile, ordered n = 8j+m."""
    a = t[:]
    return bass.AP(a.tensor, a.offset, [a.ap[0], [32, 4], [1, 8]])


def _emit(tc, io, tl, lmax, hb1, hb2):
    nc = tc.nc
    pools = []
    _scopes = []

    def S(name):
        _scopes.append((name, nc.enter_named_scope(name, False)[0]))

    def E():
        n, i = _scopes.pop()
        nc.leave_named_scope(n, i, False)

    def pool(name, bufs, space="SBUF"):
        p = tc.alloc_tile_pool(name=name, bufs=bufs, space=space)
        pools.append(p)
        return p

    maxL = max(lmax)
    nch = (maxL + 127) // 128

    const = pool("const", 1)
    state = pool("state", 1)
    gate_p = pool("gate", 2)
    ls_p = pool("ls", 2)
    xet_p = pool("xet", 3)
    g1_p = pool("g1", 2, space="PSUM")
    pstate = pool("pstate", 1, space="PSUM")

    # ---- resident constants ----
    def load_const(key, shape, dtype):
        t = const.tile(shape, dtype, name=key + "_sb")
        nc.sync.dma_start(t[:], io[key].ap())
        return t

    w1e_sb = load_const("w1e", [128, 4 * G1], F8)
    keys_sb = load_const("keys", [128, NB * T], F16)
    vals_sb = load_const("vals", [128, NCH * NB * VSP], F16)
    whh1_sb = load_const("whh1", [128, 4 * G1], F8)
    w1c_sb = load_const("w1c", [128, G1], F8)
    w2i_sb = load_const("w2i", [128, 4 * G2], F8)
    w2h_sb = load_const("w2h", [128, G2], F8)
    wout_sb = load_const("wout", [128, 2 * VOCP], F16)
    bout_sb = load_const("bout", [128, 8], F32)
    # maskind[t', c*256 + n*8 + m] = (m==n%8) * (c*128+t' < len_n)
    maskind_sb = load_const("maskind", [128, NCH * 256], F16)
    # maskNT[scat(n), t] = (t < len_n); dead rows: col 0 = 1 (denominator 1.0)
    masknt_sb = load_const("masknt", [128, 512], F16)
    if hb1:
        b1_sb = load_const("b1", [128, 16], F32)
    if hb2:
        b2_sb = load_const("b2", [128, 4], F32)

    ident_bf = const.tile([128, 128], F16, name="ident_bf")
    make_identity(nc, ident_bf[:])
    # mask8[p, n*8+m] = 1.0 iff m == n%8 (same on every partition)
    mask8 = const.tile([128, 256], F32, name="mask8")
    nc.gpsimd.memset(mask8[:], 0.0)
    nc.gpsimd.affine_select(
        out=mask8[:], in_=mask8[:],
        compare_op=ALU.not_equal, fill=1.0, base=0,
        pattern=[[0, 4], [-1, 8], [1, 8]], channel_multiplier=0,
    )
    mask8_32x8 = bass.AP(mask8[:].tensor, mask8[:].offset,
                         [mask8[:].ap[0], [8, 32], [1, 8]])

    # ---- state ----
    # gw: [i f g o | s1] (tnh output cols 0:512, S1 = 2*c1 cols 512:640)
    gw = state.tile([128, 640], F32, name="gw")
    gw2 = state.tile([128, 160], F32, name="gw2")   # [i f g o | s2], 32 each
    h1b = state.tile([128, 128], F16, name="h1b")   # H1 = 2*h1
    h2b = state.tile([128, 32], F16, name="h2b")    # H2 = 2*h2
    ctxb = state.tile([128, 128], F16, name="ctxb")  # scattered cols
    exp_t = state.tile([128, 512], F16, name="exp_t")
    attnTM = state.tile([128, NCH * 256], F16, name="attnTM")
    h2m = state.tile([128, 256], F16, name="h2m")
    rcp = state.tile([128, 1], F32, name="rcp")
    den = state.tile([128, 1], F32, name="den")
    dsc = state.tile([128, 512], F16, name="dsc")   # den scratch

    for tl_ in (gw, gw2):
        nc.vector.memset(tl_[:], 0.0)
    for tl_ in (h1b, h2b, exp_t):
        nc.vector.memset(tl_[:], 0.0)
    nc.sync.dma_start(ctxb[:], io["ctx0"].ap())

    # persistent psum tiles (1:1 bank reuse across steps; stale regions are
    # finite and get masked after exp)
    ep = pstate.tile([128, 512], F32, name="ep")
    cp = pstate.tile([128, 512], F32, name="cp")
    lp_t = pstate.tile([128, 512], F32, name="lp")
    g2p_t = pstate.tile([128, 512], F32, name="g2p")
    etp = pstate.tile([128, 1024], F16, name="etp")  # expT 0:512 | ctxT 512:640
    for tl_ in (ep, cp):
        nc.vector.memset(tl_[:], 0.0)
    # denominator col reads 1.0 on the 96 dead partitions (finite rcp)
    nc.vector.memset(cp[:, 128:129], 1.0)
    lp = lp_t[:, 0:256]
    g2p = g2p_t[:, 0:128]

    out_t = io["out"].ap().tensor     # [TL, 8, 128, 32] f16
    xemb_t = io["xemb"].ap().tensor   # [4, 128, TL*NB] f16

    def emit_xet_dma(t):
        xt = xet_p.tile([128, 4 * 32], F16, tag="xet")
        src = bass.AP(xemb_t, t * NB,
                      [[TL * NB, 128], [128 * TL * NB, 4], [1, 32]])
        nc.sync.dma_start(xt[:].rearrange("p (k b) -> p k b", k=4), src)
        return xt

    W1E_SPLIT = 9

    def emit_w1e(g1t, xt, m_lo=0, m_hi=16):
        # g1 partial: W1e @ xemb(t).  ONE start for the whole bank: the
        # pend covers all 2KB, each region's first write resets it, later
        # Whh1/W1c matmuls accumulate.
        S("w1e")
        for m in range(m_lo, m_hi):
            o = m * 32
            for k in range(4):
                nc.tensor.matmul(
                    g1t[:, o:o + 32],
                    w1e_sb[:, k * G1 + m * 128: k * G1 + (m + 1) * 128],
                    xt[:, k * 32:(k + 1) * 32],
                    start=(m == 0 and k == 0), stop=False,
                    skip_group_check=True)
        E()

    def emit_whh1(g1t):
        S("whh1")
        for m in range(16):
            o = m * 32
            for k in range(4):
                nc.tensor.matmul(
                    g1t[:, o:o + 32],
                    whh1_sb[:, k * G1 + m * 128: k * G1 + (m + 1) * 128],
                    h1b[:, k * 32:(k + 1) * 32],
                    start=False, stop=False, skip_group_check=True)
        E()

    def emit_w1c(g1t):
        S("w1c")
        ctx_rhs = _scat_ap(ctxb)
        for m in range(16):
            o = m * 32
            nc.tensor.matmul(
                g1t[:, o:o + 32], w1c_sb[:, m * 128:(m + 1) * 128],
                ctx_rhs, start=False, stop=(m == 15), skip_group_check=True)
        E()

    def pair_ap(tile_, off, stride, w):
        a = tile_[:]
        return bass.AP(a.tensor, a.offset + off, [a.ap[0], [stride, 2], [1, w]])

    # per-quadrant chain bounds: energy half-1 stop (if no half-2 in the
    # quadrant), half-2 stop; ctx first/last (c, mm) pairs
    e_stop1 = {}
    e_stop2 = {}
    for j in range(4):
        long_mm = [mm for mm in range(8) if lmax[8 * j + mm] > 256]
        e_stop1[j] = 7 if not long_mm else None
        if long_mm:
            e_stop2[j] = long_mm[-1]
    cmax = [(lmax[n] + 127) // 128 for n in range(NB)]
    c_pairs = {j: [(c, mm) for c in range(nch) for mm in range(8)
                   if c < cmax[8 * j + mm]] for j in range(4)}
    c_first = {j: p[0] for j, p in c_pairs.items()}
    c_last = {j: p[-1] for j, p in c_pairs.items()}

    # ---- prologue ----
    xcur = emit_xet_dma(0)
    xnext = emit_xet_dma(1) if tl > 1 else None
    g1cur = g1_p.tile([128, 512], F32, tag="g1")
    emit_w1e(g1cur, xcur)
    emit_whh1(g1cur)
    emit_w1c(g1cur)
    g1next = None
    if tl > 1:
        g1next = g1_p.tile([128, 512], F32, tag="g1")
        emit_w1e(g1next, xnext, 0, W1E_SPLIT)

    # ---- the recurrence ----
    for t in range(tl):
        if t + 2 < tl:
            xnext2 = emit_xet_dma(t + 2)

        # PE: rest of next step's W1e chain fills the gates1 ACT/DVE window
        # (the first W1E_SPLIT m-regions ran in the previous ctxfin window)
        if t + 1 < tl:
            emit_w1e(g1next, xnext, W1E_SPLIT, 16)

        S("gates1")
        if hb1:
            g1sb = gate_p.tile([128, 512], F32, tag="g1sb")
            b1_bc = bass.AP(b1_sb[:].tensor, b1_sb[:].offset,
                            [b1_sb[:].ap[0], [1, 16], [0, 32]])
            nc.vector.tensor_add(
                g1sb[:].rearrange("p (m b) -> p m b", m=16),
                g1cur[:].rearrange("p (m b) -> p m b", m=16), b1_bc)
            nc.scalar.activation(gw[:, 0:512], g1sb[:], AF.Tanh, scale=1.0 / WSC)
        else:
            nc.scalar.activation(gw[:, 0:512], g1cur[:], AF.Tanh, scale=1.0 / WSC)
        # a2 = (i+1)*g ; a1 = (f+1)*S1 ; S1 = a1*0.5 + a2
        aa = gate_p.tile([128, 256], F32, tag="aa")
        nc.vector.scalar_tensor_tensor(
            aa[:, 0:128], gw[:, 0:128], 1.0, gw[:, 256:384],
            ALU.add, ALU.mult)
        nc.vector.scalar_tensor_tensor(
            aa[:, 128:256], gw[:, 128:256], 1.0, gw[:, 512:640],
            ALU.add, ALU.mult)
        nc.vector.scalar_tensor_tensor(
            gw[:, 512:640], aa[:, 128:256], 0.5, aa[:, 0:128],
            ALU.mult, ALU.add)
        th = gate_p.tile([128, 128], F32, tag="th")
        nc.scalar.activation(th[:], gw[:, 512:640], AF.Tanh, scale=0.5)
        nc.vector.scalar_tensor_tensor(
            h1b[:], gw[:, 384:512], 1.0, th[:], ALU.add, ALU.mult)
        E()

        S("lstm2")
        for m in range(4):
            o = m * 32
            nc.tensor.matmul(
                g2p[:, o:o + 32], w2h_sb[:, m * 128:(m + 1) * 128],
                h2b[:], start=True, stop=False, skip_group_check=True)
            for k in range(4):
                nc.tensor.matmul(
                    g2p[:, o:o + 32],
                    w2i_sb[:, k * G2 + m * 128: k * G2 + (m + 1) * 128],
                    h1b[:, k * 32:(k + 1) * 32],
                    start=False, stop=(k == 3), skip_group_check=True)
        if hb2:
            g2sb = gate_p.tile([128, 128], F32, tag="g2sb")
            b2_bc = bass.AP(b2_sb[:].tensor, b2_sb[:].offset,
                            [b2_sb[:].ap[0], [1, 4], [0, 32]])
            nc.vector.tensor_add(
                g2sb[:].rearrange("p (m b) -> p m b", m=4),
                g2p[:].rearrange("p (m b) -> p m b", m=4), b2_bc)
            nc.scalar.activation(gw2[:, 0:128], g2sb[:], AF.Tanh, scale=1.0 / WSC)
        else:
            nc.scalar.activation(gw2[:, 0:128], g2p[:], AF.Tanh, scale=1.0 / WSC)
        aa2 = gate_p.tile([128, 64], F32, tag="aa2")
        nc.vector.scalar_tensor_tensor(
            aa2[:, 0:32], gw2[:, 0:32], 1.0, gw2[:, 64:96],
            ALU.add, ALU.mult)
        nc.vector.scalar_tensor_tensor(
            aa2[:, 32:64], gw2[:, 32:64], 1.0, gw2[:, 128:160],
            ALU.add, ALU.mult)
        nc.vector.scalar_tensor_tensor(
            gw2[:, 128:160], aa2[:, 32:64], 0.5, aa2[:, 0:32],
            ALU.mult, ALU.add)
        th2 = gate_p.tile([128, 32], F32, tag="th2")
        nc.scalar.activation(th2[:], gw2[:, 128:160], AF.Tanh, scale=0.5)
        nc.vector.scalar_tensor_tensor(
            h2b[:], gw2[:, 96:128], 1.0, th2[:], ALU.add, ALU.mult)
        for j in range(4):
            h2bc = bass.AP(h2b[:].tensor, h2b[:].offset + 8 * j,
                           [h2b[:].ap[0], [1, 8], [0, 8]])
            m8 = bass.AP(mask8[:].tensor, mask8[:].offset + 64 * j,
                         [mask8[:].ap[0], [8, 8], [1, 8]])
            nc.vector.tensor_mul(
                h2m[:, 64 * j:64 * (j + 1)].rearrange(
                    "p (n m) -> p n m", m=8), h2bc, m8)
        E()

        # PE: Whh1 @ h1(t) for step t+1 fills the lstm2 tanh/DVE window,
        # followed by the head of W1e for step t+2
        g1nn = None
        if t + 1 < tl:
            emit_whh1(g1next)
        if t + 2 < tl:
            g1nn = g1_p.tile([128, 512], F32, tag="g1")
            emit_w1e(g1nn, xnext2, 0, 4)

        S("energy")
        # one long-stream matmul per (sample, half) with one-hot-masked h2
        # columns writing that sample's psum row; the encoder axis is split
        # at col 256 so exp of the first half overlaps the tail.  Chains
        # accumulate per quadrant (column tile).
        for mm in range(8):
            for j in range(4):
                n = 8 * j + mm
                L = lmax[n]
                L1 = min(L, 256)
                nc.tensor.matmul(
                    ep[32 * j:32 * j + 8, 0:L1], h2m[:, n * 8:(n + 1) * 8],
                    keys_sb[:, n * T: n * T + L1],
                    start=(mm == 0), stop=(mm == e_stop1[j]),
                    tile_position=(0, 32 * j), skip_group_check=True)
        E()

        S("logits")
        # h2 half of the logits: independent PE work under the exp window.
        # ONE start for the whole bank -- a per-region start would re-pend
        # the earlier regions' bytes and the ctx-half accumulation below
        # would then overwrite instead of accumulate.
        for mo in range(8):
            nc.tensor.matmul(lp[:, mo * 32:(mo + 1) * 32],
                             wout_sb[:, mo * 128:(mo + 1) * 128],
                             h2b[:], start=(mo == 0), stop=False,
                             skip_group_check=True)
        E()

        S("softmax")
        nc.scalar.activation(exp_t[:, 0:128], ep[:, 0:128], AF.Exp)
        if maxL > 128:
            nc.scalar.activation(exp_t[:, 128:256], ep[:, 128:256], AF.Exp)
        E()

        S("energy2")
        for mm in range(8):
            for j in range(4):
                n = 8 * j + mm
                L = lmax[n]
                if L <= 256:
                    continue
                nc.tensor.matmul(
                    ep[32 * j:32 * j + 8, 256:L], h2m[:, n * 8:(n + 1) * 8],
                    keys_sb[:, n * T + 256: n * T + L],
                    start=False, stop=(mm == e_stop2.get(j)),
                    tile_position=(0, 32 * j), skip_group_check=True)
        E()
        if maxL > 256:
            S("softmax2")
            nc.scalar.activation(exp_t[:, 256:maxL], ep[:, 256:maxL], AF.Exp)
            E()

        S("transp")
        for c in range(nch):
            nc.tensor.transpose(etp[:, c * 128:(c + 1) * 128],
                                exp_t[:, c * 128:(c + 1) * 128], ident_bf[:])
            src = bass.AP(etp[:].tensor, etp[:].offset + c * 128,
                          [etp[:].ap[0], [32, 4], [0, 8], [1, 8]])
            nc.vector.tensor_mul(
                attnTM[:, c * 256:(c + 1) * 256].rearrange(
                    "p (j mm m) -> p j mm m", j=4, mm=8),
                src,
                maskind_sb[:, c * 256:(c + 1) * 256].rearrange(
                    "p (j mm m) -> p j mm m", j=4, mm=8))
            if c == 1 or c == nch - 1:
                # context accumulation for the chunks transposed so far
                S("ctx")
                c_lo = 0 if c <= 1 else 2
                for cc in range(c_lo, c + 1):
                    for mm in range(8):
                        for j in range(4):
                            n = 8 * j + mm
                            if cc >= cmax[n]:
                                continue
                            nc.tensor.matmul(
                                cp[32 * j:32 * j + 8, 0:VSP],
                                attnTM[:, cc * 256 + n * 8:
                                       cc * 256 + (n + 1) * 8],
                                vals_sb[:, (cc * NB + n) * VSP:
                                        (cc * NB + n + 1) * VSP],
                                start=((cc, mm) == c_first[j]),
                                stop=((cc, mm) == c_last[j]),
                                tile_position=(0, 32 * j),
                                skip_group_check=True)
                E()
        E()

        S("ctxfin")
        # early softmax denominator (off the ctx->rcp critical path):
        # den = sum_t exp * valid-mask, overlapping the ctx c23 matmuls
        nc.vector.scalar_tensor_tensor(
            dsc[:, 0:maxL], exp_t[:, 0:maxL], 1.0, masknt_sb[:, 0:maxL],
            ALU.mult, ALU.mult, accum_out=den[:])
        nc.vector.reciprocal(rcp[:], den[:])
        ctxbm = ls_p.tile([128, 128], F16, tag="ctxbm")
        nc.vector.tensor_scalar_mul(ctxbm[:], cp[:, 0:128], rcp[:])
        # W1e part-b for step t+2 fills the rcp/ctxbm window on the PE
        if t + 2 < tl:
            emit_w1e(g1nn, xnext2, 4, W1E_SPLIT)
        nc.tensor.transpose(etp[:, 512:640], ctxbm[:], ident_bf[:])
        nc.vector.tensor_copy(ctxb[:], etp[:, 512:640])
        E()

        if t + 1 < tl:
            emit_w1c(g1next)

        S("logits2")
        ctx_rhs2 = _scat_ap(ctxb)
        for mo in range(8):
            nc.tensor.matmul(lp[:, mo * 32:(mo + 1) * 32],
                             wout_sb[:, VOCP + mo * 128: VOCP + (mo + 1) * 128],
                             ctx_rhs2, start=False, stop=True,
                             skip_group_check=True)
        ls = ls_p.tile([128, 256], F16, tag="ls")
        bout_bc = bass.AP(bout_sb[:].tensor, bout_sb[:].offset,
                          [bout_sb[:].ap[0], [1, 8], [0, 32]])
        nc.vector.tensor_add(
            ls[:].rearrange("p (mo b) -> p mo b", mo=8),
            lp[:, 0:256].rearrange("p (mo b) -> p mo b", mo=8), bout_bc)
        dst = bass.AP(out_t, t * 8 * 128 * 32,
                      [[32, 128], [128 * 32, 8], [1, 32]])
        nc.gpsimd.dma_start(dst, ls[:].rearrange("p (mo b) -> p mo b", mo=8))
        E()

        if t + 1 < tl:
            g1cur = g1next
            g1next = g1nn
            xcur = xnext
            xnext = xnext2 if t + 2 < tl else None

    for p in reversed(pools):
        p.release()


@functools.lru_cache(maxsize=4)
def _build(tl, lmax, hb1, hb2):
    nc = bacc.Bacc("TRN2", target_bir_lowering=False, debug=False)
    io = {}
    io["keys"] = nc.dram_tensor("keys", [128, NB * T], F16, kind="ExternalInput")
    io["vals"] = nc.dram_tensor("vals", [128, NCH * NB * VSP], F16, kind="ExternalInput")
    io["xemb"] = nc.dram_tensor("xemb", [4, 128, TL * NB], F16, kind="ExternalInput")
    io["w1e"] = nc.dram_tensor("w1e", [128, 4 * G1], F8, kind="ExternalInput")
    io["whh1"] = nc.dram_tensor("whh1", [128, 4 * G1], F8, kind="ExternalInput")
    io["w1c"] = nc.dram_tensor("w1c", [128, G1], F8, kind="ExternalInput")
    io["w2i"] = nc.dram_tensor("w2i", [128, 4 * G2], F8, kind="ExternalInput")
    io["w2h"] = nc.dram_tensor("w2h", [128, G2], F8, kind="ExternalInput")
    io["wout"] = nc.dram_tensor("wout", [128, 2 * VOCP], F16, kind="ExternalInput")
    io["b1"] = nc.dram_tensor("b1", [128, 16], F32, kind="ExternalInput")
    io["b2"] = nc.dram_tensor("b2", [128, 4], F32, kind="ExternalInput")
    io["bout"] = nc.dram_tensor("bout", [128, 8], F32, kind="ExternalInput")
    io["ctx0"] = nc.dram_tensor("ctx0", [128, 128], F16, kind="ExternalInput")
    io["maskind"] = nc.dram_tensor("maskind", [128, NCH * 256], F16, kind="ExternalInput")
    io["masknt"] = nc.dram_tensor("masknt", [128, 512], F16, kind="ExternalInput")
    io["out"] = nc.dram_tensor("out", [TL, 8, 128, 32], F16, kind="ExternalOutput")

    with tile.TileContext(nc) as tc:
        _emit(tc, io, tl, lmax, hb1, hb2)
    nc.compile()
    return nc


def _bf(x):
    return np.asarray(x, np.float32).astype(np.float16)


def _f8(x):
    return (np.asarray(x, np.float32) * WSC).astype(ml_dtypes.float8_e4m3)


def _scat_perm():
    n = np.arange(NB)
    return 32 * (n // 8) + n % 8


def prep_inputs(key, values, lens, text, emb, W_ih1, W_hh1, b_ih1, b_hh1,
                W_ih2, W_hh2, b_ih2, b_hh2, W_out, b_out):
    key = np.asarray(key, np.float32)
    values = np.asarray(values, np.float32)
    lens = np.asarray(lens).astype(np.int64)
    text = np.asarray(text).astype(np.int64)
    emb = np.asarray(emb, np.float32)
    W_ih1 = np.asarray(W_ih1, np.float32)
    W_hh1 = np.asarray(W_hh1, np.float32)
    W_ih2 = np.asarray(W_ih2, np.float32)
    W_hh2 = np.asarray(W_hh2, np.float32)
    W_out = np.asarray(W_out, np.float32)
    b1 = np.asarray(b_ih1, np.float32) + np.asarray(b_hh1, np.float32)
    b2 = np.asarray(b_ih2, np.float32) + np.asarray(b_hh2, np.float32)
    b_out = np.asarray(b_out, np.float32)

    perm = _scat_perm()

    # sigmoid-via-tanh input scales (i, f, o rows) and the H = 2h / S = 2c
    # state-scaling compensation on consumer weights
    rs1 = np.ones((4 * H, 1), np.float32)
    rs1[0:2 * H] = 0.5          # i, f
    rs1[3 * H:4 * H] = 0.5      # o
    rs2 = np.ones((4 * KS, 1), np.float32)
    rs2[0:2 * KS] = 0.5
    rs2[3 * KS:4 * KS] = 0.5

    W1 = W_ih1 * rs1
    Wh1 = W_hh1 * rs1 * 0.5
    W2i = W_ih2 * rs2 * 0.5
    W2h = W_hh2 * rs2 * 0.5
    b1s = b1 * rs1.ravel()
    b2s = b2 * rs2.ravel()

    shared = {}
    w1T = np.ascontiguousarray(W1.T)  # (640, 2048)
    shared["w1e"] = _f8(w1T[:H].reshape(4, 128, G1).transpose(1, 0, 2).reshape(128, 4 * G1))
    shared["w1c"] = _f8(w1T[H:])
    shared["whh1"] = _f8(Wh1.T.reshape(4, 128, G1).transpose(1, 0, 2).reshape(128, 4 * G1))
    shared["w2i"] = _f8(W2i.T.reshape(4, 128, G2).transpose(1, 0, 2).reshape(128, 4 * G2))
    shared["w2h"] = _f8(np.ascontiguousarray(W2h.T))
    woutp = np.zeros((VOCP, KS + VS), np.float32)
    woutp[:VOC] = W_out
    woutp[:, :KS] *= 0.5        # h2 = H2/2
    shared["wout"] = _bf(woutp.T.reshape(2, 128, VOCP).transpose(1, 0, 2).reshape(128, 2 * VOCP))
    shared["b1"] = np.ascontiguousarray((b1s * WSC).reshape(16, 128).T)
    shared["b2"] = np.ascontiguousarray((b2s * WSC).reshape(4, 128).T)
    boutp = np.zeros((VOCP,), np.float32)
    boutp[:VOC] = b_out
    shared["bout"] = np.ascontiguousarray(boutp.reshape(8, 128).T)

    # sort batches into slots by length (ascending) per core; the slot-wise
    # max over cores defines the compile-time length profile
    lens_c = lens.reshape(NCORES, NB)
    orders = [np.argsort(lens_c[c], kind="stable") for c in range(NCORES)]
    slot_lens = np.stack([lens_c[c][orders[c]] for c in range(NCORES)])
    lmax = tuple(int(v) for v in slot_lens.max(axis=0))

    in_maps = []
    for core in range(NCORES):
        sl = slice(core * NB, (core + 1) * NB)
        order = orders[core]
        keyc = key[:, sl, :][:, order, :]
        valc = values[:, sl, :][:, order, :]
        lensc = lens[sl][order]
        textc = text[sl][order]

        m = dict(shared)
        # zero the invalid (t >= len) key rows (masked energies become 0)
        # and fold the H2 = 2*h2 compensation into the keys
        kz = keyc * 0.5 * (np.arange(T)[:, None, None] < lensc[None, :, None])
        m["keys"] = _bf(np.ascontiguousarray(
            kz.transpose(2, 1, 0)).reshape(128, NB * T))
        vp = np.zeros((NCH * 128, NB, VSP), np.float32)
        vp[:T, :, :VS] = valc
        vp[:, :, VS] = 1.0       # ones column -> softmax denominator
        m["vals"] = _bf(np.ascontiguousarray(
            vp.reshape(NCH, 128, NB * VSP).transpose(1, 0, 2)).reshape(
                128, NCH * NB * VSP))
        embs = emb[textc]                       # (32, TL, H)
        m["xemb"] = _bf(np.ascontiguousarray(
            embs.transpose(2, 1, 0)).reshape(4, 128, TL * NB))
        ctx0 = valc.mean(axis=0)                # (32, VS)
        c0 = np.zeros((128, 128), np.float32)
        c0[:, perm] = ctx0.T
        m["ctx0"] = _bf(c0)
        # maskind[t', c*256 + n*8 + m] = (m==n%8) * (c*128+t' < len_n)
        ind = (np.arange(NCH * 128)[None, :] < lensc[:, None]).astype(np.float32)
        mi = np.zeros((128, NCH, 32, 8), np.float32)
        nn = np.arange(NB)
        mi[:, :, nn, nn % 8] = ind.reshape(NB, NCH, 128).transpose(2, 1, 0)
        m["maskind"] = _bf(mi.reshape(128, NCH * 256))
        # maskNT[scat(n), t] = (t < len_n); dead rows get col0=1
        mnt = np.zeros((128, 512), np.float32)
        mnt[perm] = ind
        dead = np.ones(128, bool); dead[perm] = False
        mnt[dead, 0] = 1.0
        m["masknt"] = _bf(mnt)
        in_maps.append(m)
    return in_maps, orders, lmax, b1s, b2s


def kernel(key, values, lens, text, emb, W_ih1, W_hh1, b_ih1, b_hh1,
           W_ih2, W_hh2, b_ih2, b_hh2, W_out, b_out,
           _trace=False, _tl=TL):
    in_maps, orders, lmax, b1s, b2s = prep_inputs(
        key, values, lens, text, emb, W_ih1, W_hh1, b_ih1, b_hh1,
        W_ih2, W_hh2, b_ih2, b_hh2, W_out, b_out)
    hb1 = bool(np.any(b1s))
    hb2 = bool(np.any(b2s))
    if __import__('os').environ.get('NOTRIM'):
        lmax = tuple(T for _ in lmax)
    nc = _build(_tl, lmax, hb1, hb2)
    res = bass_utils.run_bass_kernel_spmd(
        nc, in_maps, core_ids=list(range(NCORES)), trace=_trace)
    kernel._last_results = res

    full = np.zeros((NCORES * NB, TL, VOC), np.float32)
    for core in range(NCORES):
        o = np.asarray(res.results[core]["out"]).astype(np.float32)
        o = o.reshape(TL, VOCP, 32)
        full[core * NB + orders[core]] = o[:, :VOC, :].transpose(2, 0, 1)
    return full
